# revision 24
# baseline (speedup 1.0000x reference)
"""Multi-head attention (B=4, S=2048, D=1024, H=16, causal) on 8 TRN2 NeuronCores.

Sharding: batch x head-group (Megatron).  Core c handles batch c//2 and head
group c%2 (8 heads = 512 of the 1024 hidden dims).  w_q/w_k/w_v are
column-parallel, w_o row-parallel; the two partial outputs per batch are summed
on the host during unsharding.

Device kernel (per core, all matmuls bf16, fp32 accumulation):
  - host pre-transposed x (dim-major [D, S]); wq/xq k-tiles interleaved across
    the sync+scalar DMA rings so the q-projection wavefront starts on the
    first k-tile; wk/wv/wo demoted behind on the gpsimd ring; xv streamed in
    token-chunk order so vproj(tt) unblocks per-chunk
  - qT/kT projections in [dim, token] layout; V projection in [token, dim]
    with a ones block packed next to each head's V columns
  - scoresT[k,q] = kT.T @ qT per head, two heads row-packed on the PE array
    (concurrent via row-group tiling)
  - exp on ScalarE (scores are O(1): no max subtraction needed; causal masking
    by construction: only valid k-tiles/columns computed, triangle zeroed via
    a precomputed binary mask multiply on DVE)
  - attn@V with the [V | ones] stationary tile, so the softmax denominator is
    accumulated in PSUM partitions 64:128 of the same matmul for free
  - softmax denominators inverted on DVE (reciprocal_approx_fast) instead of
    ScalarE ln/exp, keeping ScalarE exclusively on the exp stream
  - kproj(tc>0)/vproj/o-proj emitted as deadline-tracked fill work, paced
    evenly through the attention chunks so the PE never drains while ScalarE
    catches up on exp; next chunk's first scores pre-issued before the last
    attn@V of each chunk to soften seams
  - b_q added on qT evacuation, b_k dropped (cancels in softmax), b_v folded
    into b_o on host
"""

import os
import sys

for _p in ("/opt/trn_rl_repo",):
    if _p not in sys.path and os.path.isdir(_p):
        sys.path.insert(0, _p)

from contextlib import ExitStack

import ml_dtypes
import numpy as np

import concourse.bass as bass
import concourse.tile as tile
from concourse import bacc, mybir
from concourse import bass_utils

BF16 = ml_dtypes.bfloat16

B = 4
S = 2048
D = 1024
H = 16
DK = 64
NCORES = 8
DL = D // 2  # local (per head-group) hidden dims = 512
NHP = 4  # head pairs per core
KT = D // 128  # contraction tiles over model dim = 8
TT = S // 128  # token tiles = 16
QC = S // 512  # query chunks of 512 = 4

FP32 = mybir.dt.float32
DTBF = mybir.dt.bfloat16
I32 = mybir.dt.int32

# int32 magic for the bitwise reciprocal seed (one Newton step after)
RECIP_MAGIC = 0x7EF312AC

# bisect switches
USE_PREISSUE = os.environ.get("K_PREISSUE", "1") == "1"


def _emit(nc, causal: bool):
    xq = nc.dram_tensor("xq_t", [D, S], DTBF, kind="ExternalInput").ap()
    xk = nc.dram_tensor("xk_t", [D, S], DTBF, kind="ExternalInput").ap()
    xv = nc.dram_tensor("xv_t", [D, S], DTBF, kind="ExternalInput").ap()
    wq_t = nc.dram_tensor("wq_p", [128, KT, DL], DTBF, kind="ExternalInput").ap()
    wk_t = nc.dram_tensor("wk_p", [128, KT, DL], DTBF, kind="ExternalInput").ap()
    wv_t = nc.dram_tensor("wv_p", [128, KT, DL], DTBF, kind="ExternalInput").ap()
    wo_t = nc.dram_tensor("wo_p", [128, NHP, D], DTBF, kind="ExternalInput").ap()
    bq_t = nc.dram_tensor("bq_t", [128, 4], FP32, kind="ExternalInput").ap()
    bo_t = nc.dram_tensor("bo_t", [128, 8], FP32, kind="ExternalInput").ap()
    out_pt = nc.dram_tensor("out_pt", [D, S], FP32, kind="ExternalOutput").ap()

    EXP = mybir.ActivationFunctionType.Exp

    with tile.TileContext(nc) as tc, ExitStack() as ctx:
        consts = ctx.enter_context(tc.tile_pool(name="consts", bufs=1))
        xt_pool = ctx.enter_context(tc.tile_pool(name="xt", bufs=2))
        qkv_pool = ctx.enter_context(tc.tile_pool(name="qkv", bufs=1))
        et_pool = ctx.enter_context(tc.tile_pool(name="et", bufs=4))
        rc_pool = ctx.enter_context(tc.tile_pool(name="rc", bufs=4))
        out_pool = ctx.enter_context(tc.tile_pool(name="osb", bufs=3))
        ps_s = ctx.enter_context(tc.tile_pool(name="ps_s", bufs=2, space="PSUM"))
        ps_acc = ctx.enter_context(tc.tile_pool(name="ps_acc", bufs=2, space="PSUM"))
        ps_op = ctx.enter_context(tc.tile_pool(name="ps_op", bufs=2, space="PSUM"))

        # ---- input DMA schedule ---------------------------------------------
        # (wq k, xq k) pairs fan out over the THREE HWDGE rings (sync/scalar/
        # gpsimd) so the q-proj wavefront is HBM-bound, not ring-bound;
        # biases lead on gpsimd (tiny), wk/wv follow the gpsimd xq share;
        # xk ships token-chunk-major on sync/scalar (tc0 first, matching the
        # deferred per-tc k-projection fills)
        bq_sb = consts.tile([128, 4], FP32)
        nc.gpsimd.dma_start(bq_sb[:], bq_t[:])
        bo_sb = consts.tile([128, 8], FP32)
        nc.gpsimd.dma_start(bo_sb[:], bo_t[:])

        wq_sb = consts.tile([128, KT, DL], DTBF)
        xtq = xt_pool.tile([128, KT, S], DTBF, tag="xt", name="xtq")
        x_rings = [nc.sync, nc.scalar, nc.gpsimd]
        for k in range(KT):
            eng = x_rings[k % 3]
            eng.dma_start(wq_sb[:, k, :], wq_t[:, k, :])
            eng.dma_start(xtq[:, k, :], xq[k * 128:(k + 1) * 128, :])

        wk_sb = consts.tile([128, KT, DL], DTBF)
        nc.gpsimd.dma_start(wk_sb[:], wk_t[:])
        wv_sb = consts.tile([128, KT, DL], DTBF)
        nc.gpsimd.dma_start(wv_sb[:], wv_t[:])

        xtk = xt_pool.tile([128, KT, S], DTBF, tag="xt", name="xtk")
        for tc4 in range(4):
            for k in range(KT):
                eng = nc.sync if k % 2 == 0 else nc.scalar
                eng.dma_start(
                    xtk[:, k, tc4 * 512:(tc4 + 1) * 512],
                    xk[k * 128:(k + 1) * 128, tc4 * 512:(tc4 + 1) * 512],
                )

        qT_sb = qkv_pool.tile([128, NHP, S], DTBF)
        kT_sb = qkv_pool.tile([128, NHP, S], DTBF)
        # [V | ones] per head: vp_sb[:, tt, h, 0:64] = V, [.., 64:128] = 1.0
        vp_sb = qkv_pool.tile([128, TT, 8, 128], DTBF)
        a_sb = qkv_pool.tile([128, NHP, S], DTBF)

        for h in range(8):
            nc.vector.memset(vp_sb[:, :, h, 64:128], 1.0)

        # lower-triangle-inclusive (k <= q) binary mask for diagonal tiles,
        # replicated for both heads of a pair
        tri_sb = consts.tile([128, 2, 128], DTBF)
        nc.gpsimd.memset(tri_sb[:], 1.0)
        for h2 in range(2):
            nc.gpsimd.affine_select(
                out=tri_sb[:, h2, :],
                in_=tri_sb[:, h2, :],
                compare_op=mybir.AluOpType.is_ge,
                fill=0.0,
                base=0,
                pattern=[[1, 128]],
                channel_multiplier=-1,
            )

        # ---- q projection ---------------------------------------------------
        # first out-tile of q-proj in wavefront order (k outer) over the
        # not-yet-used ps_s banks: PE consumes each (wq k, xq k) pair as it
        # lands instead of waiting for the full tensor
        wf = [ps_s.tile([128, 2, 512], FP32, tag="ps_s", name=f"wf{i}") for i in range(2)]
        for k in range(KT):
            for c in range(4):
                nc.tensor.matmul(
                    wf[c // 2][:, c % 2, :],
                    wq_sb[:, k, 0:128],
                    xtq[:, k, c * 512:(c + 1) * 512],
                    start=(k == 0),
                    stop=(k == KT - 1),
                )
        for c in range(4):
            nc.vector.tensor_scalar_add(
                qT_sb[:, 0, c * 512:(c + 1) * 512], wf[c // 2][:, c % 2, :], bq_sb[:, 0:1]
            )
        for ot in range(1, 4):
            for tc4 in range(4):
                ps = ps_op.tile([128, 512], FP32, tag="op", name="ps")
                for k in range(KT):
                    nc.tensor.matmul(
                        ps[:],
                        wq_sb[:, k, ot * 128:(ot + 1) * 128],
                        xtq[:, k, tc4 * 512:(tc4 + 1) * 512],
                        start=(k == 0),
                        stop=(k == KT - 1),
                    )
                nc.vector.tensor_scalar_add(
                    qT_sb[:, ot, tc4 * 512:(tc4 + 1) * 512], ps[:], bq_sb[:, ot:ot + 1]
                )

        # xv lands last (gpsimd ring, behind wk/wv), token-half-major so
        # vproj(tt<8) needs only the first half; emission after the q-proj
        # reads of the pool buffer it rotates into.  On gpsimd (not sync or
        # scalar) because the issue instructions inherit the WAR wait on the
        # q-proj reads and would head-block the exp/out rings.  wo rides
        # behind xv — its first use (o-proj of qc0) is another ~60us out.
        xtv = xt_pool.tile([128, KT, S], DTBF, tag="xt", name="xtv")
        for half in range(2):
            for k in range(KT):
                nc.gpsimd.dma_start(
                    xtv[:, k, half * 1024:(half + 1) * 1024],
                    xv[k * 128:(k + 1) * 128, half * 1024:(half + 1) * 1024],
                )
        wo_sb = consts.tile([128, NHP, D], DTBF)
        nc.gpsimd.dma_start(wo_sb[:], wo_t[:])

        # ---- fill units -----------------------------------------------------
        def kproj_tc(ot, tc4):
            ps = ps_op.tile([128, 512], FP32, tag="op", name="ps")
            for k in range(KT):
                nc.tensor.matmul(
                    ps[:],
                    wk_sb[:, k, ot * 128:(ot + 1) * 128],
                    xtk[:, k, tc4 * 512:(tc4 + 1) * 512],
                    start=(k == 0),
                    stop=(k == KT - 1),
                )
            nc.vector.tensor_copy(kT_sb[:, ot, tc4 * 512:(tc4 + 1) * 512], ps[:])

        def vproj(tt):
            ps = ps_op.tile([128, 512], FP32, tag="op", name="ps")
            for k in range(KT):
                nc.tensor.matmul(
                    ps[:],
                    xtv[:, k, tt * 128:(tt + 1) * 128],
                    wv_sb[:, k, :],
                    start=(k == 0),
                    stop=(k == KT - 1),
                )
            nc.vector.tensor_copy(vp_sb[:, tt, :, 0:64], ps[:])

        def oproj_od(qc, od, ps_ap=None):
            if ps_ap is None:
                ps = ps_op.tile([128, 512], FP32, tag="op", name="ps")
                ps_ap = ps[:]
            for hp in range(NHP):
                nc.tensor.matmul(
                    ps_ap,
                    wo_sb[:, hp, od * 128:(od + 1) * 128],
                    a_sb[:, hp, qc * 512:(qc + 1) * 512],
                    start=(hp == 0),
                    stop=(hp == NHP - 1),
                )
            osb = out_pool.tile([128, 512], FP32, tag="osb", name="osb")
            nc.vector.tensor_scalar_add(osb[:], ps_ap, bo_sb[:, od:od + 1])
            nc.sync.dma_start(
                out_pt[od * 128:(od + 1) * 128, qc * 512:(qc + 1) * 512], osb[:]
            )

        # ---- attention pipeline ---------------------------------------------
        def offof(qc, j):
            r = j - 4 * qc if causal else -1
            return 128 * r if r >= 0 else 0

        def scores(qc, hp, j):
            off = offof(qc, j)
            q0 = qc * 512
            pss = ps_s.tile([128, 2, 512], FP32, tag="ps_s", name="pss")
            for h2 in range(2):
                nc.tensor.matmul(
                    pss[:, h2, off:512],
                    kT_sb[h2 * 64:(h2 + 1) * 64, hp, j * 128:(j + 1) * 128],
                    qT_sb[h2 * 64:(h2 + 1) * 64, hp, q0 + off:q0 + 512],
                    start=True,
                    stop=True,
                )
            et = et_pool.tile([128, 2, 512], DTBF, tag="et", name="et")
            nc.scalar.activation(et[:, :, off:], pss[:, :, off:], EXP, scale=0.125)
            if off or (causal and j == 4 * qc):
                # zero where k (partition) > q (free col), both heads
                nc.vector.tensor_mul(
                    et[:, :, off:off + 128],
                    et[:, :, off:off + 128],
                    tri_sb[:],
                )
            return et

        # fills: list of (deadline, closure); deadline = (qc, hp, j) meaning
        # the unit must be emitted before scores/attn@V of that tile, or None
        # for no deadline.  Kept deadline-sorted by construction per phase.
        fills = []
        tail_fills = []
        credit = [0.0, 0.0]  # [accumulated credit, rate per tile]

        def drain_due(key):
            while fills and fills[0][0] is not None and fills[0][0] <= key:
                fills.pop(0)[1]()

        def pop_paced():
            credit[0] += credit[1]
            while credit[0] >= 1.0 and fills:
                fills.pop(0)[1]()
                credit[0] -= 1.0

        preissued = {}
        deferred = []

        def attn(qc, hp, nxt):
            jmax = 4 * qc + 3 if causal else TT - 1
            pso = ps_acc.tile([128, 2, 512], FP32, tag="acc", name="pso", bufs=1)
            if (qc, hp) in preissued:
                et_next = preissued.pop((qc, hp))
            else:
                drain_due((qc, hp, 0))
                et_next = scores(qc, hp, 0)
            for j in range(jmax + 1):
                et = et_next
                if j < jmax:
                    drain_due((qc, hp, j + 1))
                    et_next = scores(qc, hp, j + 1)
                elif nxt is not None and USE_PREISSUE:
                    drain_due((nxt[0], nxt[1], 0))
                    preissued[nxt] = scores(nxt[0], nxt[1], 0)
                    # seam cover: the single pso accumulator is WAR-blocked
                    # until the evacuation copy drains (~1.3us); park some
                    # fill work in front of the stall
                    for _ in range(2):
                        if fills:
                            fills.pop(0)[1]()
                            credit[0] -= 1.0
                off = offof(qc, j)
                for h2 in range(2):
                    # rows 0:64 accumulate attn@V, rows 64:128 the softmax
                    # denominator (ones block).  Causally-trimmed widths on
                    # interleaved chains; per-element has_written semantics
                    # make this safe but the sim's zero-region tracker
                    # can't express it.
                    nc.tensor.matmul(
                        pso[:, h2, off:512],
                        vp_sb[:, j, 2 * hp + h2, :],
                        et[:, h2, off:],
                        start=(j == 0),
                        stop=(j == jmax),
                        skip_group_check=True,
                    )
                pop_paced()
                if deferred and j == min(2, jmax):
                    # previous chunk's normalization, deferred past this
                    # chunk's first exps so the seam exp stream is unblocked
                    deferred.pop(0)()
            if nxt is None:
                # last chunk: nothing competes for pso or the exp stream
                # anymore — normalize directly from PSUM (shortest tail)
                rl = rc_pool.tile([128, 2, 512], FP32, tag="rc", name="rl")
                rc = rc_pool.tile([128, 2, 512], FP32, tag="rc", name="rc")
                nc.scalar.activation(rl[64:128, :, :], pso[64:128, :, :],
                                     mybir.ActivationFunctionType.Ln)
                nc.scalar.activation(rc[64:128, :, :], rl[64:128, :, :], EXP, scale=-1.0)
                for h2 in range(2):
                    nc.vector.tensor_mul(
                        a_sb[h2 * 64:(h2 + 1) * 64, hp, qc * 512:(qc + 1) * 512],
                        pso[0:64, h2, :],
                        rc[64:128, h2, :],
                    )
                return
            # evacuate pso with two fast copies (numerators and denominators,
            # both landing at partitions 0:64 — PSUM inputs are exempt from
            # the SBUF same-base-partition rule) so the accumulator frees for
            # the next chunk; the ln/exp/mul tail runs deferred off the copies
            ntn = rc_pool.tile([128, 2, 512], FP32, tag="rc", name="ntn")
            ntd = rc_pool.tile([128, 2, 512], FP32, tag="rc", name="ntd")
            nc.vector.tensor_copy(ntn[0:64, :, :], pso[0:64, :, :])
            nc.vector.tensor_copy(ntd[0:64, :, :], pso[64:128, :, :])

            def norm(qc=qc, hp=hp, ntn=ntn, ntd=ntd):
                # reciprocal on the (otherwise idle) gpsimd engine: int32
                # magic-constant seed + one Newton step, ~0.26% max err --
                # keeps ScalarE exclusively on the exp stream.  t reuses
                # ntd's buffer and the result reuses u's (WAR-tracked).
                y0 = rc_pool.tile([128, 2, 512], FP32, tag="rc", name="y0")
                u = rc_pool.tile([128, 2, 512], FP32, tag="rc", name="u")
                nc.gpsimd.tensor_scalar(
                    y0[0:64, :, :].bitcast(I32), ntd[0:64, :, :].bitcast(I32),
                    RECIP_MAGIC, -1,
                    mybir.AluOpType.subtract, mybir.AluOpType.mult)
                nc.gpsimd.tensor_tensor(
                    u[0:64, :, :], ntd[0:64, :, :], y0[0:64, :, :],
                    mybir.AluOpType.mult)
                nc.gpsimd.tensor_scalar(
                    ntd[0:64, :, :], u[0:64, :, :], -1.0, 2.0,
                    mybir.AluOpType.mult, mybir.AluOpType.add)
                nc.gpsimd.tensor_tensor(
                    u[0:64, :, :], ntd[0:64, :, :], y0[0:64, :, :],
                    mybir.AluOpType.mult)
                for h2 in range(2):
                    nc.vector.tensor_mul(
                        a_sb[h2 * 64:(h2 + 1) * 64, hp, qc * 512:(qc + 1) * 512],
                        ntn[0:64, h2, :],
                        u[0:64, h2, :],
                    )

            deferred.append(norm)

        # chunk schedule: per qc, refill the fill list and set the pacing rate
        chunks = [(qc, hp) for qc in range(QC) for hp in range(NHP)]
        for ci, (qc, hp) in enumerate(chunks):
            nxt = chunks[ci + 1] if ci + 1 < len(chunks) else None
            if hp == 0:
                # phase refill at each new query chunk
                new = []
                if qc == 0:
                    # kproj: tc0 of every head-pair first (needed from chunk
                    # (0, hp)); later tcs deferred to their query chunk
                    new += [
                        ((0, op, 0), lambda op=op: kproj_tc(op, 0))
                        for op in range(NHP)
                    ]
                    first = 4 if causal else TT
                    new += [
                        ((0, 0, tt), lambda tt=tt: vproj(tt)) for tt in range(first)
                    ]
                    if not causal:
                        new += [
                            ((0, op, 4 * tc), lambda op=op, tc=tc: kproj_tc(op, tc))
                            for tc in range(1, 4)
                            for op in range(NHP)
                        ]
                else:
                    if causal:
                        new += [
                            ((qc, op, 4 * qc), lambda op=op, qc=qc: kproj_tc(op, qc))
                            for op in range(NHP)
                        ]
                        new += [
                            ((qc, 0, tt), lambda tt=tt: vproj(tt))
                            for tt in range(4 * qc, 4 * qc + 4)
                        ]
                    # o-proj fills ride two query-chunks behind their data
                    # (a_sb[qc'] is complete once chunk (qc', 3) normalizes),
                    # loading the later, ACT-heavier chunks; the last four of
                    # oproj(qc2) are reserved to keep the PE warm through the
                    # final chunk's normalize + o-proj tail
                    if qc >= 2:
                        new += [
                            (None, lambda od=od, q=qc - 2: oproj_od(q, od))
                            for od in range(8)
                        ]
                    if qc == QC - 1:
                        tail_fills.extend(
                            (None, lambda od=od, q=qc - 1: oproj_od(q, od))
                            for od in range(4, 8)
                        )
                        new += [
                            (None, lambda od=od, q=qc - 1: oproj_od(q, od))
                            for od in range(4)
                        ]
                fills.extend(new)
                fills.sort(key=lambda f: (f[0] is None, f[0] or (0, 0, 0)))
                jmax = 4 * qc + 3 if causal else TT - 1
                ntiles = NHP * (jmax + 1)
                credit[0] = 0.0
                credit[1] = len(fills) / float(ntiles)
            attn(qc, hp, nxt)
        while deferred:
            deferred.pop(0)()
        # reserved o-proj work rides the PE through the last chunk's
        # normalize so the array stays warm into the final o-proj
        while tail_fills:
            tail_fills.pop(0)[1]()
        while fills:
            fills.pop(0)[1]()

        # final chunk's o-proj: attention is over, so spread accumulators
        # across the free score banks too (6 chains in flight instead of 2)
        fin = [ps_s.tile([128, 2, 512], FP32, tag="ps_s", name=f"fin{i}") for i in range(2)]
        qc = QC - 1
        for od in range(8):
            if od < 4:
                oproj_od(qc, od, ps_ap=fin[od // 2][:, od % 2, :])
            else:
                oproj_od(qc, od)


_CACHE = {}


def _patched_act_tables(arch):
    """Make the combined Ln+Exp set the only one advertising Exp/Ln so the
    table-load pass picks it everywhere (one load, no set thrashing).  Set
    positions (= act_func_set_id) are preserved."""
    t = dict(_orig_act_tables(arch))
    name = "natural_log_exp_and_others"
    if name in t:
        exp_ln = {f for f in t[name] if f.name in ("Exp", "Ln")}
        t = {
            k: (v if k == name else (set(v) - exp_ln))
            for k, v in t.items()
        }
    return t


_orig_act_tables = bacc.get_activation_tables
bacc.get_activation_tables = _patched_act_tables


def _get_compiled(causal: bool):
    key = bool(causal)
    if key not in _CACHE:
        nc = bacc.Bacc("TRN2", target_bir_lowering=False, debug=False, num_devices=NCORES)
        _emit(nc, causal=key)
        nc.compile()
        _CACHE[key] = nc
    return _CACHE[key]


def make_in_maps(query, key, value, w_q, b_q, w_k, b_k, w_v, b_v, w_o, b_o):
    """Build the per-core input maps (host-side sharding + layout prep)."""
    in_maps = []
    # b_v folds into the output bias: softmax rows sum to 1, so
    # attn(V + b_v) = attn(V) + b_v, and (A + b_v) @ w_o.T = A @ w_o.T + w_o @ b_v.
    # b_k drops entirely: scores shift constant along k cancels in softmax.
    bo_eff = (b_o + w_o.astype(np.float64) @ b_v.astype(np.float64)).astype(np.float32)
    for c in range(NCORES):
        b, hg = divmod(c, 2)
        sl = slice(hg * DL, (hg + 1) * DL)
        bo_core = bo_eff if hg == 0 else np.zeros_like(bo_eff)
        in_maps.append(
            {
                "xq_t": np.ascontiguousarray(query[b].T).astype(BF16),
                "xk_t": np.ascontiguousarray(key[b].T).astype(BF16),
                "xv_t": np.ascontiguousarray(value[b].T).astype(BF16),
                "wq_p": np.ascontiguousarray(
                    w_q[sl, :].T.reshape(KT, 128, DL).transpose(1, 0, 2)).astype(BF16),
                "wk_p": np.ascontiguousarray(
                    w_k[sl, :].T.reshape(KT, 128, DL).transpose(1, 0, 2)).astype(BF16),
                "wv_p": np.ascontiguousarray(
                    w_v[sl, :].T.reshape(KT, 128, DL).transpose(1, 0, 2)).astype(BF16),
                "wo_p": np.ascontiguousarray(
                    w_o[:, sl].T.reshape(NHP, 128, D).transpose(1, 0, 2)).astype(BF16),
                "bq_t": np.ascontiguousarray(b_q[sl].reshape(4, 128).T).astype(np.float32),
                "bo_t": np.ascontiguousarray(bo_core.reshape(8, 128).T).astype(np.float32),
            }
        )
    return in_maps


def _mask_is_causal(mask):
    m = np.asarray(mask).reshape(S, S)
    return bool(np.array_equal(m, np.triu(np.ones((S, S), bool), k=1)))


def _mask_is_empty(mask):
    return not np.asarray(mask).any()


def kernel(query, key, value, mask, w_q, b_q, w_k, b_k, w_v, b_v, w_o, b_o, **_unused):
    query = np.asarray(query, np.float32)
    key = np.asarray(key, np.float32)
    value = np.asarray(value, np.float32)
    if _mask_is_causal(mask):
        causal = True
    elif _mask_is_empty(mask):
        causal = False
    else:
        raise NotImplementedError("only causal or empty masks are supported")

    nc = _get_compiled(causal)
    in_maps = make_in_maps(
        query, key, value,
        np.asarray(w_q, np.float32), np.asarray(b_q, np.float32),
        np.asarray(w_k, np.float32), np.asarray(b_k, np.float32),
        np.asarray(w_v, np.float32), np.asarray(b_v, np.float32),
        np.asarray(w_o, np.float32), np.asarray(b_o, np.float32),
    )
    res = bass_utils.run_bass_kernel_spmd(nc, in_maps, core_ids=list(range(NCORES)))
    out = np.empty((B, S, D), np.float32)
    for b in range(B):
        acc = res.results[2 * b] ["out_pt"] + res.results[2 * b + 1]["out_pt"]
        out[b] = acc.T
    return out


# revision 25
# speedup vs baseline: 1.6652x; 1.6652x over previous
"""Multi-head attention (B=4, S=2048, D=1024, H=16, causal) on 8 TRN2 NeuronCores.

Sharding: batch x head-group (Megatron).  Core c handles batch c//2 and head
group c%2 (8 heads = 512 of the 1024 hidden dims).  w_q/w_k/w_v are
column-parallel, w_o row-parallel; the two partial outputs per batch are summed
on the host during unsharding.

Device kernel (per core, all matmuls bf16, fp32 accumulation):
  - host pre-transposed x (dim-major [D, S]); wq/xq k-tiles interleaved across
    the sync+scalar DMA rings so the q-projection wavefront starts on the
    first k-tile; wk/wv/wo demoted behind on the gpsimd ring; xv streamed in
    token-chunk order so vproj(tt) unblocks per-chunk
  - qT/kT projections in [dim, token] layout; V projection in [token, dim]
    with a ones block packed next to each head's V columns
  - scoresT[k,q] = kT.T @ qT per head, two heads row-packed on the PE array
    (concurrent via row-group tiling)
  - exp on ScalarE (scores are O(1): no max subtraction needed; causal masking
    by construction: only valid k-tiles/columns computed, triangle zeroed via
    a precomputed binary mask multiply on DVE)
  - attn@V with the [V | ones] stationary tile, so the softmax denominator is
    accumulated in PSUM partitions 64:128 of the same matmul for free
  - softmax denominators inverted on DVE (reciprocal_approx_fast) instead of
    ScalarE ln/exp, keeping ScalarE exclusively on the exp stream
  - kproj(tc>0)/vproj/o-proj emitted as deadline-tracked fill work, paced
    evenly through the attention chunks so the PE never drains while ScalarE
    catches up on exp; next chunk's first scores pre-issued before the last
    attn@V of each chunk to soften seams
  - b_q added on qT evacuation, b_k dropped (cancels in softmax), b_v folded
    into b_o on host
"""

import os
import sys

for _p in ("/opt/trn_rl_repo",):
    if _p not in sys.path and os.path.isdir(_p):
        sys.path.insert(0, _p)

from contextlib import ExitStack

import ml_dtypes
import numpy as np

import concourse.bass as bass
import concourse.tile as tile
from concourse import bacc, mybir
from concourse import bass_utils

BF16 = ml_dtypes.bfloat16

B = 4
S = 2048
D = 1024
H = 16
DK = 64
NCORES = 8
DL = D // 2  # local (per head-group) hidden dims = 512
NHP = 4  # head pairs per core
KT = D // 128  # contraction tiles over model dim = 8
TT = S // 128  # token tiles = 16
QC = S // 512  # query chunks of 512 = 4

FP32 = mybir.dt.float32
DTBF = mybir.dt.bfloat16
I32 = mybir.dt.int32

# int32 magic for the bitwise reciprocal seed (one Newton step after)
RECIP_MAGIC = 0x7EF312AC

# bisect switches
USE_PREISSUE = os.environ.get("K_PREISSUE", "1") == "1"


def _emit(nc, causal: bool):
    xq = nc.dram_tensor("xq_t", [D, S], DTBF, kind="ExternalInput").ap()
    xk = nc.dram_tensor("xk_t", [D, S], DTBF, kind="ExternalInput").ap()
    xv = nc.dram_tensor("xv_t", [D, S], DTBF, kind="ExternalInput").ap()
    wq_t = nc.dram_tensor("wq_p", [128, KT, DL], DTBF, kind="ExternalInput").ap()
    wk_t = nc.dram_tensor("wk_p", [128, KT, DL], DTBF, kind="ExternalInput").ap()
    wv_t = nc.dram_tensor("wv_p", [128, KT, DL], DTBF, kind="ExternalInput").ap()
    wo_t = nc.dram_tensor("wo_p", [128, NHP, D], DTBF, kind="ExternalInput").ap()
    bq_t = nc.dram_tensor("bq_t", [128, 4], FP32, kind="ExternalInput").ap()
    bo_t = nc.dram_tensor("bo_t", [128, 8], FP32, kind="ExternalInput").ap()
    out_pt = nc.dram_tensor("out_pt", [D, S], FP32, kind="ExternalOutput").ap()

    EXP = mybir.ActivationFunctionType.Exp

    with tile.TileContext(nc) as tc, ExitStack() as ctx:
        consts = ctx.enter_context(tc.tile_pool(name="consts", bufs=1))
        xt_pool = ctx.enter_context(tc.tile_pool(name="xt", bufs=2))
        qkv_pool = ctx.enter_context(tc.tile_pool(name="qkv", bufs=1))
        et_pool = ctx.enter_context(tc.tile_pool(name="et", bufs=4))
        rc_pool = ctx.enter_context(tc.tile_pool(name="rc", bufs=4))
        out_pool = ctx.enter_context(tc.tile_pool(name="osb", bufs=3))
        ps_s = ctx.enter_context(tc.tile_pool(name="ps_s", bufs=2, space="PSUM"))
        ps_acc = ctx.enter_context(tc.tile_pool(name="ps_acc", bufs=2, space="PSUM"))
        ps_op = ctx.enter_context(tc.tile_pool(name="ps_op", bufs=2, space="PSUM"))

        # ---- input DMA schedule ---------------------------------------------
        # (wq k, xq k) pairs fan out over the THREE HWDGE rings (sync/scalar/
        # gpsimd) so the q-proj wavefront is HBM-bound, not ring-bound;
        # biases lead on gpsimd (tiny), wk/wv follow the gpsimd xq share;
        # xk ships token-chunk-major on sync/scalar (tc0 first, matching the
        # deferred per-tc k-projection fills)
        bq_sb = consts.tile([128, 4], FP32)
        nc.gpsimd.dma_start(bq_sb[:], bq_t[:])
        bo_sb = consts.tile([128, 8], FP32)
        nc.gpsimd.dma_start(bo_sb[:], bo_t[:])

        wq_sb = consts.tile([128, KT, DL], DTBF)
        xtq = xt_pool.tile([128, KT, S], DTBF, tag="xt", name="xtq")
        x_rings = [nc.sync, nc.scalar, nc.gpsimd]
        for k in range(KT):
            eng = x_rings[k % 3]
            eng.dma_start(wq_sb[:, k, :], wq_t[:, k, :])
            eng.dma_start(xtq[:, k, :], xq[k * 128:(k + 1) * 128, :])

        wk_sb = consts.tile([128, KT, DL], DTBF)
        nc.gpsimd.dma_start(wk_sb[:], wk_t[:])
        wv_sb = consts.tile([128, KT, DL], DTBF)
        nc.gpsimd.dma_start(wv_sb[:], wv_t[:])

        xtk = xt_pool.tile([128, KT, S], DTBF, tag="xt", name="xtk")
        for tc4 in range(4):
            for k in range(KT):
                eng = nc.sync if k % 2 == 0 else nc.scalar
                eng.dma_start(
                    xtk[:, k, tc4 * 512:(tc4 + 1) * 512],
                    xk[k * 128:(k + 1) * 128, tc4 * 512:(tc4 + 1) * 512],
                )

        qT_sb = qkv_pool.tile([128, NHP, S], DTBF)
        kT_sb = qkv_pool.tile([128, NHP, S], DTBF)
        # [V | ones] per head: vp_sb[:, tt, h, 0:64] = V, [.., 64:128] = 1.0
        vp_sb = qkv_pool.tile([128, TT, 8, 128], DTBF)
        a_sb = qkv_pool.tile([128, NHP, S], DTBF)

        for h in range(8):
            nc.vector.memset(vp_sb[:, :, h, 64:128], 1.0)

        # lower-triangle-inclusive (k <= q) binary mask for diagonal tiles,
        # replicated for both heads of a pair
        tri_sb = consts.tile([128, 2, 128], DTBF)
        nc.gpsimd.memset(tri_sb[:], 1.0)
        for h2 in range(2):
            nc.gpsimd.affine_select(
                out=tri_sb[:, h2, :],
                in_=tri_sb[:, h2, :],
                compare_op=mybir.AluOpType.is_ge,
                fill=0.0,
                base=0,
                pattern=[[1, 128]],
                channel_multiplier=-1,
            )

        # ---- q projection ---------------------------------------------------
        # first out-tile of q-proj in wavefront order (k outer) over the
        # not-yet-used ps_s banks: PE consumes each (wq k, xq k) pair as it
        # lands instead of waiting for the full tensor
        wf = [ps_s.tile([128, 2, 512], FP32, tag="ps_s", name=f"wf{i}") for i in range(2)]
        for k in range(KT):
            for c in range(4):
                nc.tensor.matmul(
                    wf[c // 2][:, c % 2, :],
                    wq_sb[:, k, 0:128],
                    xtq[:, k, c * 512:(c + 1) * 512],
                    start=(k == 0),
                    stop=(k == KT - 1),
                )
        for c in range(4):
            nc.vector.tensor_scalar_add(
                qT_sb[:, 0, c * 512:(c + 1) * 512], wf[c // 2][:, c % 2, :], bq_sb[:, 0:1]
            )
        for ot in range(1, 4):
            for tc4 in range(4):
                ps = ps_op.tile([128, 512], FP32, tag="op", name="ps")
                for k in range(KT):
                    nc.tensor.matmul(
                        ps[:],
                        wq_sb[:, k, ot * 128:(ot + 1) * 128],
                        xtq[:, k, tc4 * 512:(tc4 + 1) * 512],
                        start=(k == 0),
                        stop=(k == KT - 1),
                    )
                nc.vector.tensor_scalar_add(
                    qT_sb[:, ot, tc4 * 512:(tc4 + 1) * 512], ps[:], bq_sb[:, ot:ot + 1]
                )

        # xv lands last (gpsimd ring, behind wk/wv), token-half-major so
        # vproj(tt<8) needs only the first half; emission after the q-proj
        # reads of the pool buffer it rotates into.  On gpsimd (not sync or
        # scalar) because the issue instructions inherit the WAR wait on the
        # q-proj reads and would head-block the exp/out rings.  wo rides
        # behind xv — its first use (o-proj of qc0) is another ~60us out.
        xtv = xt_pool.tile([128, KT, S], DTBF, tag="xt", name="xtv")
        for half in range(2):
            for k in range(KT):
                nc.gpsimd.dma_start(
                    xtv[:, k, half * 1024:(half + 1) * 1024],
                    xv[k * 128:(k + 1) * 128, half * 1024:(half + 1) * 1024],
                )
        wo_sb = consts.tile([128, NHP, D], DTBF)
        nc.gpsimd.dma_start(wo_sb[:], wo_t[:])

        # ---- fill units -----------------------------------------------------
        def kproj_tc(ot, tc4):
            ps = ps_op.tile([128, 512], FP32, tag="op", name="ps")
            for k in range(KT):
                nc.tensor.matmul(
                    ps[:],
                    wk_sb[:, k, ot * 128:(ot + 1) * 128],
                    xtk[:, k, tc4 * 512:(tc4 + 1) * 512],
                    start=(k == 0),
                    stop=(k == KT - 1),
                )
            nc.vector.tensor_copy(kT_sb[:, ot, tc4 * 512:(tc4 + 1) * 512], ps[:])

        def vproj(tt):
            ps = ps_op.tile([128, 512], FP32, tag="op", name="ps")
            for k in range(KT):
                nc.tensor.matmul(
                    ps[:],
                    xtv[:, k, tt * 128:(tt + 1) * 128],
                    wv_sb[:, k, :],
                    start=(k == 0),
                    stop=(k == KT - 1),
                )
            nc.vector.tensor_copy(vp_sb[:, tt, :, 0:64], ps[:])

        def oproj_od(qc, od, ps_ap=None):
            if ps_ap is None:
                ps = ps_op.tile([128, 512], FP32, tag="op", name="ps")
                ps_ap = ps[:]
            for hp in range(NHP):
                nc.tensor.matmul(
                    ps_ap,
                    wo_sb[:, hp, od * 128:(od + 1) * 128],
                    a_sb[:, hp, qc * 512:(qc + 1) * 512],
                    start=(hp == 0),
                    stop=(hp == NHP - 1),
                )
            osb = out_pool.tile([128, 512], FP32, tag="osb", name="osb")
            nc.vector.tensor_scalar_add(osb[:], ps_ap, bo_sb[:, od:od + 1])
            nc.sync.dma_start(
                out_pt[od * 128:(od + 1) * 128, qc * 512:(qc + 1) * 512], osb[:]
            )

        # ---- attention pipeline ---------------------------------------------
        def offof(qc, j):
            r = j - 4 * qc if causal else -1
            return 128 * r if r >= 0 else 0

        def scores(qc, hp, j):
            off = offof(qc, j)
            q0 = qc * 512
            pss = ps_s.tile([128, 2, 512], FP32, tag="ps_s", name="pss")
            for h2 in range(2):
                nc.tensor.matmul(
                    pss[:, h2, off:512],
                    kT_sb[h2 * 64:(h2 + 1) * 64, hp, j * 128:(j + 1) * 128],
                    qT_sb[h2 * 64:(h2 + 1) * 64, hp, q0 + off:q0 + 512],
                    start=True,
                    stop=True,
                )
            et = et_pool.tile([128, 2, 512], DTBF, tag="et", name="et")
            nc.scalar.activation(et[:, :, off:], pss[:, :, off:], EXP, scale=0.125)
            if off or (causal and j == 4 * qc):
                # zero where k (partition) > q (free col), both heads
                nc.vector.tensor_mul(
                    et[:, :, off:off + 128],
                    et[:, :, off:off + 128],
                    tri_sb[:],
                )
            return et

        # fills: list of (deadline, closure); deadline = (qc, hp, j) meaning
        # the unit must be emitted before scores/attn@V of that tile, or None
        # for no deadline.  Kept deadline-sorted by construction per phase.
        fills = []
        tail_fills = []
        credit = [0.0, 0.0]  # [accumulated credit, rate per tile]

        def drain_due(key):
            while fills and fills[0][0] is not None and fills[0][0] <= key:
                fills.pop(0)[1]()

        def pop_paced():
            credit[0] += credit[1]
            while credit[0] >= 1.0 and fills:
                fills.pop(0)[1]()
                credit[0] -= 1.0

        preissued = {}
        deferred = []

        def attn(qc, hp, nxt):
            jmax = 4 * qc + 3 if causal else TT - 1
            pso = ps_acc.tile([128, 2, 512], FP32, tag="acc", name="pso", bufs=1)
            if (qc, hp) in preissued:
                et_next = preissued.pop((qc, hp))
            else:
                drain_due((qc, hp, 0))
                et_next = scores(qc, hp, 0)
            for j in range(jmax + 1):
                et = et_next
                if j < jmax:
                    drain_due((qc, hp, j + 1))
                    et_next = scores(qc, hp, j + 1)
                elif nxt is not None and USE_PREISSUE:
                    drain_due((nxt[0], nxt[1], 0))
                    preissued[nxt] = scores(nxt[0], nxt[1], 0)
                    # seam cover: the single pso accumulator is WAR-blocked
                    # until the evacuation copy drains (~1.3us); park some
                    # fill work in front of the stall
                    for _ in range(2):
                        if fills:
                            fills.pop(0)[1]()
                            credit[0] -= 1.0
                off = offof(qc, j)
                for h2 in range(2):
                    # rows 0:64 accumulate attn@V, rows 64:128 the softmax
                    # denominator (ones block).  Causally-trimmed widths on
                    # interleaved chains; per-element has_written semantics
                    # make this safe but the sim's zero-region tracker
                    # can't express it.
                    nc.tensor.matmul(
                        pso[:, h2, off:512],
                        vp_sb[:, j, 2 * hp + h2, :],
                        et[:, h2, off:],
                        start=(j == 0),
                        stop=(j == jmax),
                        skip_group_check=True,
                    )
                pop_paced()
                if deferred and j == min(2, jmax):
                    # previous chunk's normalization, deferred past this
                    # chunk's first exps so the seam exp stream is unblocked
                    deferred.pop(0)()
            if nxt is None:
                # last chunk: nothing competes for pso or the exp stream
                # anymore — normalize directly from PSUM (shortest tail)
                rl = rc_pool.tile([128, 2, 512], FP32, tag="rc", name="rl")
                rc = rc_pool.tile([128, 2, 512], FP32, tag="rc", name="rc")
                nc.scalar.activation(rl[64:128, :, :], pso[64:128, :, :],
                                     mybir.ActivationFunctionType.Ln)
                nc.scalar.activation(rc[64:128, :, :], rl[64:128, :, :], EXP, scale=-1.0)
                for h2 in range(2):
                    nc.vector.tensor_mul(
                        a_sb[h2 * 64:(h2 + 1) * 64, hp, qc * 512:(qc + 1) * 512],
                        pso[0:64, h2, :],
                        rc[64:128, h2, :],
                    )
                return
            # evacuate pso with two fast copies (numerators and denominators,
            # both landing at partitions 0:64 — PSUM inputs are exempt from
            # the SBUF same-base-partition rule) so the accumulator frees for
            # the next chunk; the ln/exp/mul tail runs deferred off the copies
            ntn = rc_pool.tile([128, 2, 512], FP32, tag="rc", name="ntn")
            ntd = rc_pool.tile([128, 2, 512], FP32, tag="rc", name="ntd")
            nc.vector.tensor_copy(ntn[0:64, :, :], pso[0:64, :, :])
            nc.vector.tensor_copy(ntd[0:64, :, :], pso[64:128, :, :])

            def norm(qc=qc, hp=hp, ntn=ntn, ntd=ntd):
                # reciprocal on the (otherwise idle) gpsimd engine: int32
                # magic-constant seed + one Newton step, ~0.26% max err --
                # keeps ScalarE exclusively on the exp stream.  t reuses
                # ntd's buffer and the result reuses u's (WAR-tracked).
                y0 = rc_pool.tile([128, 2, 512], FP32, tag="rc", name="y0")
                u = rc_pool.tile([128, 2, 512], FP32, tag="rc", name="u")
                # int32 on the Q7 firmware is ~15us/op; the seed stays on DVE
                nc.vector.tensor_scalar(
                    y0[0:64, :, :].bitcast(I32), ntd[0:64, :, :].bitcast(I32),
                    RECIP_MAGIC, -1,
                    mybir.AluOpType.subtract, mybir.AluOpType.mult)
                nc.gpsimd.tensor_tensor(
                    u[0:64, :, :], ntd[0:64, :, :], y0[0:64, :, :],
                    mybir.AluOpType.mult)
                nc.gpsimd.tensor_scalar(
                    ntd[0:64, :, :], u[0:64, :, :], -1.0, 2.0,
                    mybir.AluOpType.mult, mybir.AluOpType.add)
                nc.gpsimd.tensor_tensor(
                    u[0:64, :, :], ntd[0:64, :, :], y0[0:64, :, :],
                    mybir.AluOpType.mult)
                for h2 in range(2):
                    nc.vector.tensor_mul(
                        a_sb[h2 * 64:(h2 + 1) * 64, hp, qc * 512:(qc + 1) * 512],
                        ntn[0:64, h2, :],
                        u[0:64, h2, :],
                    )

            deferred.append(norm)

        # chunk schedule: per qc, refill the fill list and set the pacing rate
        chunks = [(qc, hp) for qc in range(QC) for hp in range(NHP)]
        for ci, (qc, hp) in enumerate(chunks):
            nxt = chunks[ci + 1] if ci + 1 < len(chunks) else None
            if hp == 0:
                # phase refill at each new query chunk
                new = []
                if qc == 0:
                    # kproj: tc0 of every head-pair first (needed from chunk
                    # (0, hp)); later tcs deferred to their query chunk
                    new += [
                        ((0, op, 0), lambda op=op: kproj_tc(op, 0))
                        for op in range(NHP)
                    ]
                    first = 4 if causal else TT
                    new += [
                        ((0, 0, tt), lambda tt=tt: vproj(tt)) for tt in range(first)
                    ]
                    if not causal:
                        new += [
                            ((0, op, 4 * tc), lambda op=op, tc=tc: kproj_tc(op, tc))
                            for tc in range(1, 4)
                            for op in range(NHP)
                        ]
                else:
                    if causal:
                        new += [
                            ((qc, op, 4 * qc), lambda op=op, qc=qc: kproj_tc(op, qc))
                            for op in range(NHP)
                        ]
                        new += [
                            ((qc, 0, tt), lambda tt=tt: vproj(tt))
                            for tt in range(4 * qc, 4 * qc + 4)
                        ]
                    # o-proj fills ride two query-chunks behind their data
                    # (a_sb[qc'] is complete once chunk (qc', 3) normalizes),
                    # loading the later, ACT-heavier chunks; the last four of
                    # oproj(qc2) are reserved to keep the PE warm through the
                    # final chunk's normalize + o-proj tail
                    if qc >= 2:
                        new += [
                            (None, lambda od=od, q=qc - 2: oproj_od(q, od))
                            for od in range(8)
                        ]
                    if qc == QC - 1:
                        tail_fills.extend(
                            (None, lambda od=od, q=qc - 1: oproj_od(q, od))
                            for od in range(4, 8)
                        )
                        new += [
                            (None, lambda od=od, q=qc - 1: oproj_od(q, od))
                            for od in range(4)
                        ]
                fills.extend(new)
                fills.sort(key=lambda f: (f[0] is None, f[0] or (0, 0, 0)))
                jmax = 4 * qc + 3 if causal else TT - 1
                ntiles = NHP * (jmax + 1)
                credit[0] = 0.0
                credit[1] = len(fills) / float(ntiles)
            attn(qc, hp, nxt)
        while deferred:
            deferred.pop(0)()
        # reserved o-proj work rides the PE through the last chunk's
        # normalize so the array stays warm into the final o-proj
        while tail_fills:
            tail_fills.pop(0)[1]()
        while fills:
            fills.pop(0)[1]()

        # final chunk's o-proj: attention is over, so spread accumulators
        # across the free score banks too (6 chains in flight instead of 2)
        fin = [ps_s.tile([128, 2, 512], FP32, tag="ps_s", name=f"fin{i}") for i in range(2)]
        qc = QC - 1
        for od in range(8):
            if od < 4:
                oproj_od(qc, od, ps_ap=fin[od // 2][:, od % 2, :])
            else:
                oproj_od(qc, od)


_CACHE = {}


def _patched_act_tables(arch):
    """Make the combined Ln+Exp set the only one advertising Exp/Ln so the
    table-load pass picks it everywhere (one load, no set thrashing).  Set
    positions (= act_func_set_id) are preserved."""
    t = dict(_orig_act_tables(arch))
    name = "natural_log_exp_and_others"
    if name in t:
        exp_ln = {f for f in t[name] if f.name in ("Exp", "Ln")}
        t = {
            k: (v if k == name else (set(v) - exp_ln))
            for k, v in t.items()
        }
    return t


_orig_act_tables = bacc.get_activation_tables
bacc.get_activation_tables = _patched_act_tables


def _get_compiled(causal: bool):
    key = bool(causal)
    if key not in _CACHE:
        nc = bacc.Bacc("TRN2", target_bir_lowering=False, debug=False, num_devices=NCORES)
        _emit(nc, causal=key)
        nc.compile()
        _CACHE[key] = nc
    return _CACHE[key]


def make_in_maps(query, key, value, w_q, b_q, w_k, b_k, w_v, b_v, w_o, b_o):
    """Build the per-core input maps (host-side sharding + layout prep)."""
    in_maps = []
    # b_v folds into the output bias: softmax rows sum to 1, so
    # attn(V + b_v) = attn(V) + b_v, and (A + b_v) @ w_o.T = A @ w_o.T + w_o @ b_v.
    # b_k drops entirely: scores shift constant along k cancels in softmax.
    bo_eff = (b_o + w_o.astype(np.float64) @ b_v.astype(np.float64)).astype(np.float32)
    for c in range(NCORES):
        b, hg = divmod(c, 2)
        sl = slice(hg * DL, (hg + 1) * DL)
        bo_core = bo_eff if hg == 0 else np.zeros_like(bo_eff)
        in_maps.append(
            {
                "xq_t": np.ascontiguousarray(query[b].T).astype(BF16),
                "xk_t": np.ascontiguousarray(key[b].T).astype(BF16),
                "xv_t": np.ascontiguousarray(value[b].T).astype(BF16),
                "wq_p": np.ascontiguousarray(
                    w_q[sl, :].T.reshape(KT, 128, DL).transpose(1, 0, 2)).astype(BF16),
                "wk_p": np.ascontiguousarray(
                    w_k[sl, :].T.reshape(KT, 128, DL).transpose(1, 0, 2)).astype(BF16),
                "wv_p": np.ascontiguousarray(
                    w_v[sl, :].T.reshape(KT, 128, DL).transpose(1, 0, 2)).astype(BF16),
                "wo_p": np.ascontiguousarray(
                    w_o[:, sl].T.reshape(NHP, 128, D).transpose(1, 0, 2)).astype(BF16),
                "bq_t": np.ascontiguousarray(b_q[sl].reshape(4, 128).T).astype(np.float32),
                "bo_t": np.ascontiguousarray(bo_core.reshape(8, 128).T).astype(np.float32),
            }
        )
    return in_maps


def _mask_is_causal(mask):
    m = np.asarray(mask).reshape(S, S)
    return bool(np.array_equal(m, np.triu(np.ones((S, S), bool), k=1)))


def _mask_is_empty(mask):
    return not np.asarray(mask).any()


def kernel(query, key, value, mask, w_q, b_q, w_k, b_k, w_v, b_v, w_o, b_o, **_unused):
    query = np.asarray(query, np.float32)
    key = np.asarray(key, np.float32)
    value = np.asarray(value, np.float32)
    if _mask_is_causal(mask):
        causal = True
    elif _mask_is_empty(mask):
        causal = False
    else:
        raise NotImplementedError("only causal or empty masks are supported")

    nc = _get_compiled(causal)
    in_maps = make_in_maps(
        query, key, value,
        np.asarray(w_q, np.float32), np.asarray(b_q, np.float32),
        np.asarray(w_k, np.float32), np.asarray(b_k, np.float32),
        np.asarray(w_v, np.float32), np.asarray(b_v, np.float32),
        np.asarray(w_o, np.float32), np.asarray(b_o, np.float32),
    )
    res = bass_utils.run_bass_kernel_spmd(nc, in_maps, core_ids=list(range(NCORES)))
    out = np.empty((B, S, D), np.float32)
    for b in range(B):
        acc = res.results[2 * b] ["out_pt"] + res.results[2 * b + 1]["out_pt"]
        out[b] = acc.T
    return out


# revision 26
# speedup vs baseline: 1.7663x; 1.0607x over previous
"""Multi-head attention (B=4, S=2048, D=1024, H=16, causal) on 8 TRN2 NeuronCores.

Sharding: batch x head-group (Megatron).  Core c handles batch c//2 and head
group c%2 (8 heads = 512 of the 1024 hidden dims).  w_q/w_k/w_v are
column-parallel, w_o row-parallel; the two partial outputs per batch are summed
on the host during unsharding.

Device kernel (per core, all matmuls bf16, fp32 accumulation):
  - host pre-transposed x (dim-major [D, S]); wq/xq k-tiles interleaved across
    the sync+scalar DMA rings so the q-projection wavefront starts on the
    first k-tile; wk/wv/wo demoted behind on the gpsimd ring; xv streamed in
    token-chunk order so vproj(tt) unblocks per-chunk
  - qT/kT projections in [dim, token] layout; V projection in [token, dim]
    with a ones block packed next to each head's V columns
  - scoresT[k,q] = kT.T @ qT per head, two heads row-packed on the PE array
    (concurrent via row-group tiling)
  - exp on ScalarE (scores are O(1): no max subtraction needed; causal masking
    by construction: only valid k-tiles/columns computed, triangle zeroed via
    a precomputed binary mask multiply on DVE)
  - attn@V with the [V | ones] stationary tile, so the softmax denominator is
    accumulated in PSUM partitions 64:128 of the same matmul for free
  - softmax denominators inverted on DVE (reciprocal_approx_fast) instead of
    ScalarE ln/exp, keeping ScalarE exclusively on the exp stream
  - kproj(tc>0)/vproj/o-proj emitted as deadline-tracked fill work, paced
    evenly through the attention chunks so the PE never drains while ScalarE
    catches up on exp; next chunk's first scores pre-issued before the last
    attn@V of each chunk to soften seams
  - b_q added on qT evacuation, b_k dropped (cancels in softmax), b_v folded
    into b_o on host
"""

import os
import sys

for _p in ("/opt/trn_rl_repo",):
    if _p not in sys.path and os.path.isdir(_p):
        sys.path.insert(0, _p)

from contextlib import ExitStack

import ml_dtypes
import numpy as np

import concourse.bass as bass
import concourse.tile as tile
from concourse import bacc, mybir
from concourse import bass_utils

BF16 = ml_dtypes.bfloat16

B = 4
S = 2048
D = 1024
H = 16
DK = 64
NCORES = 8
DL = D // 2  # local (per head-group) hidden dims = 512
NHP = 4  # head pairs per core
KT = D // 128  # contraction tiles over model dim = 8
TT = S // 128  # token tiles = 16
QC = S // 512  # query chunks of 512 = 4

FP32 = mybir.dt.float32
DTBF = mybir.dt.bfloat16
I32 = mybir.dt.int32

# int32 magic for the bitwise reciprocal seed (one Newton step after)
RECIP_MAGIC = 0x7EF312AC

# bisect switches
USE_PREISSUE = os.environ.get("K_PREISSUE", "1") == "1"


def _emit(nc, causal: bool):
    xq = nc.dram_tensor("xq_t", [D, S], DTBF, kind="ExternalInput").ap()
    xk = nc.dram_tensor("xk_t", [D, S], DTBF, kind="ExternalInput").ap()
    xv = nc.dram_tensor("xv_t", [D, S], DTBF, kind="ExternalInput").ap()
    wq_t = nc.dram_tensor("wq_p", [128, KT, DL], DTBF, kind="ExternalInput").ap()
    wk_t = nc.dram_tensor("wk_p", [128, KT, DL], DTBF, kind="ExternalInput").ap()
    wv_t = nc.dram_tensor("wv_p", [128, KT, DL], DTBF, kind="ExternalInput").ap()
    wo_t = nc.dram_tensor("wo_p", [128, NHP, D], DTBF, kind="ExternalInput").ap()
    bq_t = nc.dram_tensor("bq_t", [128, 4], FP32, kind="ExternalInput").ap()
    bo_t = nc.dram_tensor("bo_t", [128, 8], FP32, kind="ExternalInput").ap()
    out_pt = nc.dram_tensor("out_pt", [D, S], FP32, kind="ExternalOutput").ap()

    EXP = mybir.ActivationFunctionType.Exp

    with tile.TileContext(nc) as tc, ExitStack() as ctx:
        consts = ctx.enter_context(tc.tile_pool(name="consts", bufs=1))
        xt_pool = ctx.enter_context(tc.tile_pool(name="xt", bufs=2))
        qkv_pool = ctx.enter_context(tc.tile_pool(name="qkv", bufs=1))
        et_pool = ctx.enter_context(tc.tile_pool(name="et", bufs=4))
        rc_pool = ctx.enter_context(tc.tile_pool(name="rc", bufs=4))
        out_pool = ctx.enter_context(tc.tile_pool(name="osb", bufs=3))
        ps_s = ctx.enter_context(tc.tile_pool(name="ps_s", bufs=2, space="PSUM"))
        ps_acc = ctx.enter_context(tc.tile_pool(name="ps_acc", bufs=2, space="PSUM"))
        ps_op = ctx.enter_context(tc.tile_pool(name="ps_op", bufs=2, space="PSUM"))

        # ---- input DMA schedule ---------------------------------------------
        # (wq k, xq k) pairs fan out over the THREE HWDGE rings (sync/scalar/
        # gpsimd) so the q-proj wavefront is HBM-bound, not ring-bound;
        # biases lead on gpsimd (tiny), wk/wv follow the gpsimd xq share;
        # xk ships token-chunk-major on sync/scalar (tc0 first, matching the
        # deferred per-tc k-projection fills)
        bq_sb = consts.tile([128, 4], FP32)
        nc.gpsimd.dma_start(bq_sb[:], bq_t[:])
        bo_sb = consts.tile([128, 8], FP32)
        nc.gpsimd.dma_start(bo_sb[:], bo_t[:])

        wq_sb = consts.tile([128, KT, DL], DTBF)
        xtq = xt_pool.tile([128, KT, S], DTBF, tag="xt", name="xtq")
        x_rings = [nc.sync, nc.scalar, nc.gpsimd]
        for k in range(KT):
            eng = x_rings[k % 3]
            eng.dma_start(wq_sb[:, k, :], wq_t[:, k, :])
            eng.dma_start(xtq[:, k, :], xq[k * 128:(k + 1) * 128, :])

        wk_sb = consts.tile([128, KT, DL], DTBF)
        nc.gpsimd.dma_start(wk_sb[:], wk_t[:])
        wv_sb = consts.tile([128, KT, DL], DTBF)
        nc.gpsimd.dma_start(wv_sb[:], wv_t[:])

        xtk = xt_pool.tile([128, KT, S], DTBF, tag="xt", name="xtk")
        for tc4 in range(4):
            for k in range(KT):
                eng = nc.sync if k % 2 == 0 else nc.scalar
                eng.dma_start(
                    xtk[:, k, tc4 * 512:(tc4 + 1) * 512],
                    xk[k * 128:(k + 1) * 128, tc4 * 512:(tc4 + 1) * 512],
                )

        qT_sb = qkv_pool.tile([128, NHP, S], DTBF)
        kT_sb = qkv_pool.tile([128, NHP, S], DTBF)
        # [V | ones] per head: vp_sb[:, tt, h, 0:64] = V, [.., 64:128] = 1.0
        vp_sb = qkv_pool.tile([128, TT, 8, 128], DTBF)
        a_sb = qkv_pool.tile([128, NHP, S], DTBF)

        for h in range(8):
            nc.vector.memset(vp_sb[:, :, h, 64:128], 1.0)

        # lower-triangle-inclusive (k <= q) binary mask for diagonal tiles,
        # replicated for both heads of a pair
        tri_sb = consts.tile([128, 2, 128], DTBF)
        nc.gpsimd.memset(tri_sb[:], 1.0)
        for h2 in range(2):
            nc.gpsimd.affine_select(
                out=tri_sb[:, h2, :],
                in_=tri_sb[:, h2, :],
                compare_op=mybir.AluOpType.is_ge,
                fill=0.0,
                base=0,
                pattern=[[1, 128]],
                channel_multiplier=-1,
            )

        # ---- q projection ---------------------------------------------------
        # first out-tile of q-proj in wavefront order (k outer) over the
        # not-yet-used ps_s banks: PE consumes each (wq k, xq k) pair as it
        # lands instead of waiting for the full tensor
        wf = [ps_s.tile([128, 2, 512], FP32, tag="ps_s", name=f"wf{i}") for i in range(2)]
        for k in range(KT):
            for c in range(4):
                nc.tensor.matmul(
                    wf[c // 2][:, c % 2, :],
                    wq_sb[:, k, 0:128],
                    xtq[:, k, c * 512:(c + 1) * 512],
                    start=(k == 0),
                    stop=(k == KT - 1),
                )
        for c in range(4):
            nc.vector.tensor_scalar_add(
                qT_sb[:, 0, c * 512:(c + 1) * 512], wf[c // 2][:, c % 2, :], bq_sb[:, 0:1]
            )
        for ot in range(1, 4):
            for tc4 in range(4):
                ps = ps_op.tile([128, 512], FP32, tag="op", name="ps")
                for k in range(KT):
                    nc.tensor.matmul(
                        ps[:],
                        wq_sb[:, k, ot * 128:(ot + 1) * 128],
                        xtq[:, k, tc4 * 512:(tc4 + 1) * 512],
                        start=(k == 0),
                        stop=(k == KT - 1),
                    )
                nc.vector.tensor_scalar_add(
                    qT_sb[:, ot, tc4 * 512:(tc4 + 1) * 512], ps[:], bq_sb[:, ot:ot + 1]
                )

        # xv lands last (gpsimd ring, behind wk/wv), token-half-major so
        # vproj(tt<8) needs only the first half; emission after the q-proj
        # reads of the pool buffer it rotates into.  On gpsimd (not sync or
        # scalar) because the issue instructions inherit the WAR wait on the
        # q-proj reads and would head-block the exp/out rings.  wo rides
        # behind xv — its first use (o-proj of qc0) is another ~60us out.
        xtv = xt_pool.tile([128, KT, S], DTBF, tag="xt", name="xtv")
        for half in range(2):
            for k in range(KT):
                nc.gpsimd.dma_start(
                    xtv[:, k, half * 1024:(half + 1) * 1024],
                    xv[k * 128:(k + 1) * 128, half * 1024:(half + 1) * 1024],
                )
        wo_sb = consts.tile([128, NHP, D], DTBF)
        nc.gpsimd.dma_start(wo_sb[:], wo_t[:])

        # ---- fill units -----------------------------------------------------
        def kproj_tc(ot, tc4):
            ps = ps_op.tile([128, 512], FP32, tag="op", name="ps")
            for k in range(KT):
                nc.tensor.matmul(
                    ps[:],
                    wk_sb[:, k, ot * 128:(ot + 1) * 128],
                    xtk[:, k, tc4 * 512:(tc4 + 1) * 512],
                    start=(k == 0),
                    stop=(k == KT - 1),
                )
            nc.vector.tensor_copy(kT_sb[:, ot, tc4 * 512:(tc4 + 1) * 512], ps[:])

        def vproj(tt):
            ps = ps_op.tile([128, 512], FP32, tag="op", name="ps")
            for k in range(KT):
                nc.tensor.matmul(
                    ps[:],
                    xtv[:, k, tt * 128:(tt + 1) * 128],
                    wv_sb[:, k, :],
                    start=(k == 0),
                    stop=(k == KT - 1),
                )
            nc.vector.tensor_copy(vp_sb[:, tt, :, 0:64], ps[:])

        def oproj_od(qc, od, ps_ap=None):
            if ps_ap is None:
                ps = ps_op.tile([128, 512], FP32, tag="op", name="ps")
                ps_ap = ps[:]
            for hp in range(NHP):
                nc.tensor.matmul(
                    ps_ap,
                    wo_sb[:, hp, od * 128:(od + 1) * 128],
                    a_sb[:, hp, qc * 512:(qc + 1) * 512],
                    start=(hp == 0),
                    stop=(hp == NHP - 1),
                )
            osb = out_pool.tile([128, 512], FP32, tag="osb", name="osb")
            nc.vector.tensor_scalar_add(osb[:], ps_ap, bo_sb[:, od:od + 1])
            nc.sync.dma_start(
                out_pt[od * 128:(od + 1) * 128, qc * 512:(qc + 1) * 512], osb[:]
            )

        # ---- attention pipeline ---------------------------------------------
        def offof(qc, j):
            r = j - 4 * qc if causal else -1
            return 128 * r if r >= 0 else 0

        def scores(qc, hp, j):
            off = offof(qc, j)
            q0 = qc * 512
            pss = ps_s.tile([128, 2, 512], FP32, tag="ps_s", name="pss")
            for h2 in range(2):
                nc.tensor.matmul(
                    pss[:, h2, off:512],
                    kT_sb[h2 * 64:(h2 + 1) * 64, hp, j * 128:(j + 1) * 128],
                    qT_sb[h2 * 64:(h2 + 1) * 64, hp, q0 + off:q0 + 512],
                    start=True,
                    stop=True,
                )
            et = et_pool.tile([128, 2, 512], DTBF, tag="et", name="et")
            nc.scalar.activation(et[:, :, off:], pss[:, :, off:], EXP, scale=0.125)
            if off or (causal and j == 4 * qc):
                # zero where k (partition) > q (free col), both heads
                nc.vector.tensor_mul(
                    et[:, :, off:off + 128],
                    et[:, :, off:off + 128],
                    tri_sb[:],
                )
            return et

        # fills: list of (deadline, closure); deadline = (qc, hp, j) meaning
        # the unit must be emitted before scores/attn@V of that tile, or None
        # for no deadline.  Kept deadline-sorted by construction per phase.
        fills = []
        tail_fills = []
        credit = [0.0, 0.0]  # [accumulated credit, rate per tile]

        def drain_due(key):
            while fills and fills[0][0] is not None and fills[0][0] <= key:
                fills.pop(0)[1]()

        def pop_paced():
            credit[0] += credit[1]
            while credit[0] >= 1.0 and fills:
                fills.pop(0)[1]()
                credit[0] -= 1.0

        preissued = {}
        deferred = []

        def attn(qc, hp, nxt):
            jmax = 4 * qc + 3 if causal else TT - 1
            pso = ps_acc.tile([128, 2, 512], FP32, tag="acc", name="pso", bufs=1)
            if (qc, hp) in preissued:
                et_next = preissued.pop((qc, hp))
            else:
                drain_due((qc, hp, 0))
                et_next = scores(qc, hp, 0)
            for j in range(jmax + 1):
                et = et_next
                if j < jmax:
                    drain_due((qc, hp, j + 1))
                    et_next = scores(qc, hp, j + 1)
                elif nxt is not None and USE_PREISSUE:
                    drain_due((nxt[0], nxt[1], 0))
                    preissued[nxt] = scores(nxt[0], nxt[1], 0)
                    # seam cover: the single pso accumulator is WAR-blocked
                    # until the evacuation copy drains (~1.3us); park some
                    # fill work in front of the stall
                    for _ in range(2):
                        if fills:
                            fills.pop(0)[1]()
                            credit[0] -= 1.0
                off = offof(qc, j)
                for h2 in range(2):
                    # rows 0:64 accumulate attn@V, rows 64:128 the softmax
                    # denominator (ones block).  Causally-trimmed widths on
                    # interleaved chains; per-element has_written semantics
                    # make this safe but the sim's zero-region tracker
                    # can't express it.
                    nc.tensor.matmul(
                        pso[:, h2, off:512],
                        vp_sb[:, j, 2 * hp + h2, :],
                        et[:, h2, off:],
                        start=(j == 0),
                        stop=(j == jmax),
                        skip_group_check=True,
                    )
                pop_paced()
                if deferred and j == min(2, jmax):
                    # previous chunk's normalization, deferred past this
                    # chunk's first exps so the seam exp stream is unblocked
                    deferred.pop(0)()
            if nxt is None:
                # last chunk: nothing competes for pso or the exp stream
                # anymore — normalize directly from PSUM (shortest tail)
                rl = rc_pool.tile([128, 2, 512], FP32, tag="rc", name="rl")
                rc = rc_pool.tile([128, 2, 512], FP32, tag="rc", name="rc")
                nc.scalar.activation(rl[64:128, :, :], pso[64:128, :, :],
                                     mybir.ActivationFunctionType.Ln)
                nc.scalar.activation(rc[64:128, :, :], rl[64:128, :, :], EXP, scale=-1.0)
                for h2 in range(2):
                    nc.vector.tensor_mul(
                        a_sb[h2 * 64:(h2 + 1) * 64, hp, qc * 512:(qc + 1) * 512],
                        pso[0:64, h2, :],
                        rc[64:128, h2, :],
                    )
                return
            # evacuate pso with two fast copies (numerators and denominators,
            # both landing at partitions 0:64 — PSUM inputs are exempt from
            # the SBUF same-base-partition rule) so the accumulator frees for
            # the next chunk; the ln/exp/mul tail runs deferred off the copies
            ntn = rc_pool.tile([128, 2, 512], FP32, tag="rc", name="ntn")
            ntd = rc_pool.tile([128, 2, 512], FP32, tag="rc", name="ntd")
            nc.vector.tensor_copy(ntn[0:64, :, :], pso[0:64, :, :])
            nc.vector.tensor_copy(ntd[0:64, :, :], pso[64:128, :, :])

            def norm(qc=qc, hp=hp, ntn=ntn, ntd=ntd):
                rl = rc_pool.tile([128, 2, 512], FP32, tag="rc", name="rl")
                rc = rc_pool.tile([128, 2, 512], FP32, tag="rc", name="rc")
                nc.scalar.activation(rl[0:64, :, :], ntd[0:64, :, :],
                                     mybir.ActivationFunctionType.Ln)
                nc.scalar.activation(rc[0:64, :, :], rl[0:64, :, :], EXP, scale=-1.0)
                for h2 in range(2):
                    nc.vector.tensor_mul(
                        a_sb[h2 * 64:(h2 + 1) * 64, hp, qc * 512:(qc + 1) * 512],
                        ntn[0:64, h2, :],
                        rc[0:64, h2, :],
                    )

            deferred.append(norm)

        # chunk schedule: per qc, refill the fill list and set the pacing rate
        chunks = [(qc, hp) for qc in range(QC) for hp in range(NHP)]
        for ci, (qc, hp) in enumerate(chunks):
            nxt = chunks[ci + 1] if ci + 1 < len(chunks) else None
            if hp == 0:
                # phase refill at each new query chunk
                new = []
                if qc == 0:
                    # kproj: tc0 of every head-pair first (needed from chunk
                    # (0, hp)); later tcs deferred to their query chunk
                    new += [
                        ((0, op, 0), lambda op=op: kproj_tc(op, 0))
                        for op in range(NHP)
                    ]
                    first = 4 if causal else TT
                    new += [
                        ((0, 0, tt), lambda tt=tt: vproj(tt)) for tt in range(first)
                    ]
                    if not causal:
                        new += [
                            ((0, op, 4 * tc), lambda op=op, tc=tc: kproj_tc(op, tc))
                            for tc in range(1, 4)
                            for op in range(NHP)
                        ]
                else:
                    if causal:
                        new += [
                            ((qc, op, 4 * qc), lambda op=op, qc=qc: kproj_tc(op, qc))
                            for op in range(NHP)
                        ]
                        new += [
                            ((qc, 0, tt), lambda tt=tt: vproj(tt))
                            for tt in range(4 * qc, 4 * qc + 4)
                        ]
                    # o-proj fills ride two query-chunks behind their data
                    # (a_sb[qc'] is complete once chunk (qc', 3) normalizes),
                    # loading the later, ACT-heavier chunks; the last four of
                    # oproj(qc2) are reserved to keep the PE warm through the
                    # final chunk's normalize + o-proj tail
                    if qc >= 2:
                        new += [
                            (None, lambda od=od, q=qc - 2: oproj_od(q, od))
                            for od in range(8)
                        ]
                    if qc == QC - 1:
                        tail_fills.extend(
                            (None, lambda od=od, q=qc - 1: oproj_od(q, od))
                            for od in range(4, 8)
                        )
                        new += [
                            (None, lambda od=od, q=qc - 1: oproj_od(q, od))
                            for od in range(4)
                        ]
                fills.extend(new)
                fills.sort(key=lambda f: (f[0] is None, f[0] or (0, 0, 0)))
                jmax = 4 * qc + 3 if causal else TT - 1
                ntiles = NHP * (jmax + 1)
                credit[0] = 0.0
                credit[1] = len(fills) / float(ntiles)
            attn(qc, hp, nxt)
        while deferred:
            deferred.pop(0)()
        # reserved o-proj work rides the PE through the last chunk's
        # normalize so the array stays warm into the final o-proj
        while tail_fills:
            tail_fills.pop(0)[1]()
        while fills:
            fills.pop(0)[1]()

        # final chunk's o-proj: attention is over, so spread accumulators
        # across the free score banks too (6 chains in flight instead of 2)
        fin = [ps_s.tile([128, 2, 512], FP32, tag="ps_s", name=f"fin{i}") for i in range(2)]
        qc = QC - 1
        for od in range(8):
            if od < 4:
                oproj_od(qc, od, ps_ap=fin[od // 2][:, od % 2, :])
            else:
                oproj_od(qc, od)


_CACHE = {}


def _patched_act_tables(arch):
    """Make the combined Ln+Exp set the only one advertising Exp/Ln so the
    table-load pass picks it everywhere (one load, no set thrashing).  Set
    positions (= act_func_set_id) are preserved."""
    t = dict(_orig_act_tables(arch))
    name = "natural_log_exp_and_others"
    if name in t:
        exp_ln = {f for f in t[name] if f.name in ("Exp", "Ln")}
        t = {
            k: (v if k == name else (set(v) - exp_ln))
            for k, v in t.items()
        }
    return t


_orig_act_tables = bacc.get_activation_tables
bacc.get_activation_tables = _patched_act_tables


def _get_compiled(causal: bool):
    key = bool(causal)
    if key not in _CACHE:
        nc = bacc.Bacc("TRN2", target_bir_lowering=False, debug=False, num_devices=NCORES)
        _emit(nc, causal=key)
        nc.compile()
        _CACHE[key] = nc
    return _CACHE[key]


def make_in_maps(query, key, value, w_q, b_q, w_k, b_k, w_v, b_v, w_o, b_o):
    """Build the per-core input maps (host-side sharding + layout prep)."""
    in_maps = []
    # b_v folds into the output bias: softmax rows sum to 1, so
    # attn(V + b_v) = attn(V) + b_v, and (A + b_v) @ w_o.T = A @ w_o.T + w_o @ b_v.
    # b_k drops entirely: scores shift constant along k cancels in softmax.
    bo_eff = (b_o + w_o.astype(np.float64) @ b_v.astype(np.float64)).astype(np.float32)
    for c in range(NCORES):
        b, hg = divmod(c, 2)
        sl = slice(hg * DL, (hg + 1) * DL)
        bo_core = bo_eff if hg == 0 else np.zeros_like(bo_eff)
        in_maps.append(
            {
                "xq_t": np.ascontiguousarray(query[b].T).astype(BF16),
                "xk_t": np.ascontiguousarray(key[b].T).astype(BF16),
                "xv_t": np.ascontiguousarray(value[b].T).astype(BF16),
                "wq_p": np.ascontiguousarray(
                    w_q[sl, :].T.reshape(KT, 128, DL).transpose(1, 0, 2)).astype(BF16),
                "wk_p": np.ascontiguousarray(
                    w_k[sl, :].T.reshape(KT, 128, DL).transpose(1, 0, 2)).astype(BF16),
                "wv_p": np.ascontiguousarray(
                    w_v[sl, :].T.reshape(KT, 128, DL).transpose(1, 0, 2)).astype(BF16),
                "wo_p": np.ascontiguousarray(
                    w_o[:, sl].T.reshape(NHP, 128, D).transpose(1, 0, 2)).astype(BF16),
                "bq_t": np.ascontiguousarray(b_q[sl].reshape(4, 128).T).astype(np.float32),
                "bo_t": np.ascontiguousarray(bo_core.reshape(8, 128).T).astype(np.float32),
            }
        )
    return in_maps


def _mask_is_causal(mask):
    m = np.asarray(mask).reshape(S, S)
    return bool(np.array_equal(m, np.triu(np.ones((S, S), bool), k=1)))


def _mask_is_empty(mask):
    return not np.asarray(mask).any()


def kernel(query, key, value, mask, w_q, b_q, w_k, b_k, w_v, b_v, w_o, b_o, **_unused):
    query = np.asarray(query, np.float32)
    key = np.asarray(key, np.float32)
    value = np.asarray(value, np.float32)
    if _mask_is_causal(mask):
        causal = True
    elif _mask_is_empty(mask):
        causal = False
    else:
        raise NotImplementedError("only causal or empty masks are supported")

    nc = _get_compiled(causal)
    in_maps = make_in_maps(
        query, key, value,
        np.asarray(w_q, np.float32), np.asarray(b_q, np.float32),
        np.asarray(w_k, np.float32), np.asarray(b_k, np.float32),
        np.asarray(w_v, np.float32), np.asarray(b_v, np.float32),
        np.asarray(w_o, np.float32), np.asarray(b_o, np.float32),
    )
    res = bass_utils.run_bass_kernel_spmd(nc, in_maps, core_ids=list(range(NCORES)))
    out = np.empty((B, S, D), np.float32)
    for b in range(B):
        acc = res.results[2 * b] ["out_pt"] + res.results[2 * b + 1]["out_pt"]
        out[b] = acc.T
    return out


# revision 31
# speedup vs baseline: 1.8507x; 1.0478x over previous
"""Multi-head attention (B=4, S=2048, D=1024, H=16, causal) on 8 TRN2 NeuronCores.

Sharding: batch x head-group (Megatron).  Core c handles batch c//2 and head
group c%2 (8 heads = 512 of the 1024 hidden dims).  w_q/w_k/w_v are
column-parallel, w_o row-parallel; the two partial outputs per batch are summed
on the host during unsharding.

Device kernel (per core, all matmuls bf16, fp32 accumulation):
  - host pre-transposed x (dim-major [D, S]); wq/xq k-tiles interleaved across
    the sync+scalar DMA rings so the q-projection wavefront starts on the
    first k-tile; wk/wv/wo demoted behind on the gpsimd ring; xv streamed in
    token-chunk order so vproj(tt) unblocks per-chunk
  - qT/kT projections in [dim, token] layout; V projection in [token, dim]
    with a ones block packed next to each head's V columns
  - scoresT[k,q] = kT.T @ qT per head, two heads row-packed on the PE array
    (concurrent via row-group tiling)
  - exp on ScalarE (scores are O(1): no max subtraction needed; causal masking
    by construction: only valid k-tiles/columns computed, triangle zeroed via
    a precomputed binary mask multiply on DVE)
  - attn@V with the [V | ones] stationary tile, so the softmax denominator is
    accumulated in PSUM partitions 64:128 of the same matmul for free
  - softmax denominators inverted on DVE (reciprocal_approx_fast) instead of
    ScalarE ln/exp, keeping ScalarE exclusively on the exp stream
  - kproj(tc>0)/vproj/o-proj emitted as deadline-tracked fill work, paced
    evenly through the attention chunks so the PE never drains while ScalarE
    catches up on exp; next chunk's first scores pre-issued before the last
    attn@V of each chunk to soften seams
  - b_q added on qT evacuation, b_k dropped (cancels in softmax), b_v folded
    into b_o on host
"""

import os
import sys

for _p in ("/opt/trn_rl_repo",):
    if _p not in sys.path and os.path.isdir(_p):
        sys.path.insert(0, _p)

from contextlib import ExitStack

import ml_dtypes
import numpy as np

import concourse.bass as bass
import concourse.tile as tile
from concourse import bacc, mybir
from concourse import bass_utils

BF16 = ml_dtypes.bfloat16

B = 4
S = 2048
D = 1024
H = 16
DK = 64
NCORES = 8
DL = D // 2  # local (per head-group) hidden dims = 512
NHP = 4  # head pairs per core
KT = D // 128  # contraction tiles over model dim = 8
TT = S // 128  # token tiles = 16
QC = S // 512  # query chunks of 512 = 4

FP32 = mybir.dt.float32
DTBF = mybir.dt.bfloat16
I32 = mybir.dt.int32

# int32 magic for the bitwise reciprocal seed (one Newton step after)
RECIP_MAGIC = 0x7EF312AC

# bisect switches
USE_PREISSUE = os.environ.get("K_PREISSUE", "1") == "1"


def _emit(nc, causal: bool):
    xq = nc.dram_tensor("xq_t", [D, S], DTBF, kind="ExternalInput").ap()
    xk = nc.dram_tensor("xk_t", [D, S], DTBF, kind="ExternalInput").ap()
    xv = nc.dram_tensor("xv_t", [D, S], DTBF, kind="ExternalInput").ap()
    wq_t = nc.dram_tensor("wq_p", [128, KT, DL], DTBF, kind="ExternalInput").ap()
    wk_t = nc.dram_tensor("wk_p", [128, KT, DL], DTBF, kind="ExternalInput").ap()
    wv_t = nc.dram_tensor("wv_p", [128, KT, DL], DTBF, kind="ExternalInput").ap()
    wo_t = nc.dram_tensor("wo_p", [128, NHP, D], DTBF, kind="ExternalInput").ap()
    bq_t = nc.dram_tensor("bq_t", [128, 4], FP32, kind="ExternalInput").ap()
    bo_t = nc.dram_tensor("bo_t", [128, 8], FP32, kind="ExternalInput").ap()
    out_pt = nc.dram_tensor("out_pt", [D, S], FP32, kind="ExternalOutput").ap()

    EXP = mybir.ActivationFunctionType.Exp

    with tile.TileContext(nc) as tc, ExitStack() as ctx:
        consts = ctx.enter_context(tc.tile_pool(name="consts", bufs=1))
        xt_pool = ctx.enter_context(tc.tile_pool(name="xt", bufs=2))
        qkv_pool = ctx.enter_context(tc.tile_pool(name="qkv", bufs=1))
        et_pool = ctx.enter_context(tc.tile_pool(name="et", bufs=4))
        rc_pool = ctx.enter_context(tc.tile_pool(name="rc", bufs=4))
        out_pool = ctx.enter_context(tc.tile_pool(name="osb", bufs=3))
        ps_s = ctx.enter_context(tc.tile_pool(name="ps_s", bufs=2, space="PSUM"))
        ps_acc = ctx.enter_context(tc.tile_pool(name="ps_acc", bufs=2, space="PSUM"))
        ps_op = ctx.enter_context(tc.tile_pool(name="ps_op", bufs=2, space="PSUM"))

        # ---- input DMA schedule ---------------------------------------------
        # (wq k, xq k) pairs fan out over the THREE HWDGE rings (sync/scalar/
        # gpsimd) so the q-proj wavefront is HBM-bound, not ring-bound;
        # biases lead on gpsimd (tiny), wk/wv follow the gpsimd xq share;
        # xk ships token-chunk-major on sync/scalar (tc0 first, matching the
        # deferred per-tc k-projection fills)
        bq_sb = consts.tile([128, 4], FP32)
        nc.gpsimd.dma_start(bq_sb[:], bq_t[:])
        bo_sb = consts.tile([128, 8], FP32)
        nc.gpsimd.dma_start(bo_sb[:], bo_t[:])

        wq_sb = consts.tile([128, KT, DL], DTBF)
        xtq = xt_pool.tile([128, KT, S], DTBF, tag="xt", name="xtq")
        x_rings = [nc.sync, nc.scalar, nc.gpsimd]
        for k in range(KT):
            eng = x_rings[k % 3]
            eng.dma_start(wq_sb[:, k, :], wq_t[:, k, :])
            eng.dma_start(xtq[:, k, :], xq[k * 128:(k + 1) * 128, :])

        wk_sb = consts.tile([128, KT, DL], DTBF)
        nc.gpsimd.dma_start(wk_sb[:], wk_t[:])
        wv_sb = consts.tile([128, KT, DL], DTBF)
        nc.gpsimd.dma_start(wv_sb[:], wv_t[:])

        xtk = xt_pool.tile([128, KT, S], DTBF, tag="xt", name="xtk")
        for tc4 in range(4):
            for k in range(KT):
                eng = nc.sync if k % 2 == 0 else nc.scalar
                eng.dma_start(
                    xtk[:, k, tc4 * 512:(tc4 + 1) * 512],
                    xk[k * 128:(k + 1) * 128, tc4 * 512:(tc4 + 1) * 512],
                )

        qT_sb = qkv_pool.tile([128, NHP, S], DTBF)
        kT_sb = qkv_pool.tile([128, NHP, S], DTBF)
        # [V | ones] per head: vp_sb[:, tt, h, 0:64] = V, [.., 64:128] = 1.0
        vp_sb = qkv_pool.tile([128, TT, 8, 128], DTBF)
        a_sb = qkv_pool.tile([128, NHP, S], DTBF)

        for h in range(8):
            nc.vector.memset(vp_sb[:, :, h, 64:128], 1.0)

        # lower-triangle-inclusive (k <= q) binary mask for diagonal tiles,
        # replicated for both heads of a pair
        tri_sb = consts.tile([128, 2, 128], DTBF)
        nc.gpsimd.memset(tri_sb[:], 1.0)
        for h2 in range(2):
            nc.gpsimd.affine_select(
                out=tri_sb[:, h2, :],
                in_=tri_sb[:, h2, :],
                compare_op=mybir.AluOpType.is_ge,
                fill=0.0,
                base=0,
                pattern=[[1, 128]],
                channel_multiplier=-1,
            )

        # ---- q projection ---------------------------------------------------
        # first TWO out-tiles of q-proj in wavefront order (k outer) across
        # all eight free PSUM banks: the PE does ~1.7us of work per (wq k,
        # xq k) DMA pair -- matching the pair arrival rate -- instead of
        # idling half the DMA-paced stream
        wf = [ps_s.tile([128, 2, 512], FP32, tag="ps_s", name=f"wf{i}") for i in range(2)]
        wfb = ps_acc.tile([128, 2, 512], FP32, tag="acc", name="wfb", bufs=1)
        wfo = [ps_op.tile([128, 512], FP32, tag="op", name=f"wfo{i}") for i in range(2)]

        def _wf_target(ot, c):
            if ot == 0:
                return wf[c // 2][:, c % 2, :]
            return wfb[:, c, :] if c < 2 else wfo[c - 2][:]

        for k in range(KT):
            for ot in range(2):
                for c in range(4):
                    nc.tensor.matmul(
                        _wf_target(ot, c),
                        wq_sb[:, k, ot * 128:(ot + 1) * 128],
                        xtq[:, k, c * 512:(c + 1) * 512],
                        start=(k == 0),
                        stop=(k == KT - 1),
                    )
        for ot in range(2):
            for c in range(4):
                nc.vector.tensor_scalar_add(
                    qT_sb[:, ot, c * 512:(c + 1) * 512], _wf_target(ot, c),
                    bq_sb[:, ot:ot + 1]
                )
        for ot in range(2, 4):
            for tc4 in range(4):
                ps = ps_op.tile([128, 512], FP32, tag="op", name="ps")
                for k in range(KT):
                    nc.tensor.matmul(
                        ps[:],
                        wq_sb[:, k, ot * 128:(ot + 1) * 128],
                        xtq[:, k, tc4 * 512:(tc4 + 1) * 512],
                        start=(k == 0),
                        stop=(k == KT - 1),
                    )
                nc.vector.tensor_scalar_add(
                    qT_sb[:, ot, tc4 * 512:(tc4 + 1) * 512], ps[:], bq_sb[:, ot:ot + 1]
                )

        # xv lands last (gpsimd ring, behind wk/wv), token-half-major so
        # vproj(tt<8) needs only the first half; emission after the q-proj
        # reads of the pool buffer it rotates into.  On gpsimd (not sync or
        # scalar) because the issue instructions inherit the WAR wait on the
        # q-proj reads and would head-block the exp/out rings.  wo rides
        # behind xv — its first use (o-proj of qc0) is another ~60us out.
        xtv = xt_pool.tile([128, KT, S], DTBF, tag="xt", name="xtv")
        for half in range(2):
            for k in range(KT):
                nc.gpsimd.dma_start(
                    xtv[:, k, half * 1024:(half + 1) * 1024],
                    xv[k * 128:(k + 1) * 128, half * 1024:(half + 1) * 1024],
                )
        wo_sb = consts.tile([128, NHP, D], DTBF)
        nc.gpsimd.dma_start(wo_sb[:], wo_t[:])

        # ---- fill units -----------------------------------------------------
        def kproj_tc(ot, tc4):
            ps = ps_op.tile([128, 512], FP32, tag="op", name="ps")
            for k in range(KT):
                nc.tensor.matmul(
                    ps[:],
                    wk_sb[:, k, ot * 128:(ot + 1) * 128],
                    xtk[:, k, tc4 * 512:(tc4 + 1) * 512],
                    start=(k == 0),
                    stop=(k == KT - 1),
                )
            nc.vector.tensor_copy(kT_sb[:, ot, tc4 * 512:(tc4 + 1) * 512], ps[:])

        def vproj(tt):
            ps = ps_op.tile([128, 512], FP32, tag="op", name="ps")
            for k in range(KT):
                nc.tensor.matmul(
                    ps[:],
                    xtv[:, k, tt * 128:(tt + 1) * 128],
                    wv_sb[:, k, :],
                    start=(k == 0),
                    stop=(k == KT - 1),
                )
            nc.vector.tensor_copy(vp_sb[:, tt, :, 0:64], ps[:])

        def oproj_od(qc, od, ps_ap=None):
            if ps_ap is None:
                ps = ps_op.tile([128, 512], FP32, tag="op", name="ps")
                ps_ap = ps[:]
            for hp in range(NHP):
                nc.tensor.matmul(
                    ps_ap,
                    wo_sb[:, hp, od * 128:(od + 1) * 128],
                    a_sb[:, hp, qc * 512:(qc + 1) * 512],
                    start=(hp == 0),
                    stop=(hp == NHP - 1),
                )
            osb = out_pool.tile([128, 512], FP32, tag="osb", name="osb")
            nc.vector.tensor_scalar_add(osb[:], ps_ap, bo_sb[:, od:od + 1])
            nc.sync.dma_start(
                out_pt[od * 128:(od + 1) * 128, qc * 512:(qc + 1) * 512], osb[:]
            )

        # ---- attention pipeline ---------------------------------------------
        def offof(qc, j):
            r = j - 4 * qc if causal else -1
            return 128 * r if r >= 0 else 0

        def scores(qc, hp, j):
            off = offof(qc, j)
            q0 = qc * 512
            pss = ps_s.tile([128, 2, 512], FP32, tag="ps_s", name="pss")
            for h2 in range(2):
                nc.tensor.matmul(
                    pss[:, h2, off:512],
                    kT_sb[h2 * 64:(h2 + 1) * 64, hp, j * 128:(j + 1) * 128],
                    qT_sb[h2 * 64:(h2 + 1) * 64, hp, q0 + off:q0 + 512],
                    start=True,
                    stop=True,
                )
            et = et_pool.tile([128, 2, 512], DTBF, tag="et", name="et")
            nc.scalar.activation(et[:, :, off:], pss[:, :, off:], EXP, scale=0.125)
            if off or (causal and j == 4 * qc):
                # zero where k (partition) > q (free col), both heads
                nc.vector.tensor_mul(
                    et[:, :, off:off + 128],
                    et[:, :, off:off + 128],
                    tri_sb[:],
                )
            return et

        # fills: list of (deadline, closure); deadline = (qc, hp, j) meaning
        # the unit must be emitted before scores/attn@V of that tile, or None
        # for no deadline.  Kept deadline-sorted by construction per phase.
        fills = []
        tail_fills = []
        credit = [0.0, 0.0]  # [accumulated credit, rate per tile]

        def drain_due(key):
            while fills and fills[0][0] is not None and fills[0][0] <= key:
                fills.pop(0)[1]()

        def pop_paced():
            credit[0] += credit[1]
            while credit[0] >= 1.0 and fills:
                fills.pop(0)[1]()
                credit[0] -= 1.0

        preissued = {}
        deferred = []

        def attn(qc, hp, nxt):
            jmax = 4 * qc + 3 if causal else TT - 1
            pso = ps_acc.tile([128, 2, 512], FP32, tag="acc", name="pso", bufs=1)
            if (qc, hp) in preissued:
                et_next = preissued.pop((qc, hp))
            else:
                drain_due((qc, hp, 0))
                et_next = scores(qc, hp, 0)
            for j in range(jmax + 1):
                et = et_next
                if j < jmax:
                    drain_due((qc, hp, j + 1))
                    et_next = scores(qc, hp, j + 1)
                elif nxt is not None and USE_PREISSUE:
                    drain_due((nxt[0], nxt[1], 0))
                    preissued[nxt] = scores(nxt[0], nxt[1], 0)
                    # seam cover: the single pso accumulator is WAR-blocked
                    # until the evacuation copy drains (~1.3us); park some
                    # fill work in front of the stall
                    for _ in range(2):
                        if fills:
                            fills.pop(0)[1]()
                            credit[0] -= 1.0
                off = offof(qc, j)
                for h2 in range(2):
                    # rows 0:64 accumulate attn@V, rows 64:128 the softmax
                    # denominator (ones block).  Causally-trimmed widths on
                    # interleaved chains; per-element has_written semantics
                    # make this safe but the sim's zero-region tracker
                    # can't express it.
                    nc.tensor.matmul(
                        pso[:, h2, off:512],
                        vp_sb[:, j, 2 * hp + h2, :],
                        et[:, h2, off:],
                        start=(j == 0),
                        stop=(j == jmax),
                        skip_group_check=True,
                    )
                pop_paced()
                # previous chunk's normalization, deferred past this chunk's
                # first exps (and spread in small units so no op head-blocks
                # an engine queue)
                while deferred and deferred[0][0] <= j:
                    deferred.pop(0)[1]()
            if nxt is None:
                # last chunk: nothing competes for pso or the exp stream
                # anymore — normalize directly from PSUM (shortest tail)
                rl = rc_pool.tile([128, 2, 512], FP32, tag="rc", name="rl")
                rc = rc_pool.tile([128, 2, 512], FP32, tag="rc", name="rc")
                nc.scalar.activation(rl[64:128, :, :], pso[64:128, :, :],
                                     mybir.ActivationFunctionType.Ln)
                nc.scalar.activation(rc[64:128, :, :], rl[64:128, :, :], EXP, scale=-1.0)
                for h2 in range(2):
                    nc.vector.tensor_mul(
                        a_sb[h2 * 64:(h2 + 1) * 64, hp, qc * 512:(qc + 1) * 512],
                        pso[0:64, h2, :],
                        rc[64:128, h2, :],
                    )
                return
            # evacuate pso with two fast copies (numerators and denominators,
            # both landing at partitions 0:64 — PSUM inputs are exempt from
            # the SBUF same-base-partition rule) so the accumulator frees for
            # the next chunk; the ln/exp/mul tail runs deferred off the copies
            ntn = rc_pool.tile([128, 2, 512], FP32, tag="rc", name="ntn")
            ntd = rc_pool.tile([128, 2, 512], FP32, tag="rc", name="ntd")
            nc.vector.tensor_copy(ntn[0:64, :, :], pso[0:64, :, :])
            nc.vector.tensor_copy(ntd[0:64, :, :], pso[64:128, :, :])

            def mul_unit(h2, rcap, qc=qc, hp=hp, ntn=ntn):
                nc.vector.tensor_mul(
                    a_sb[h2 * 64:(h2 + 1) * 64, hp, qc * 512:(qc + 1) * 512],
                    ntn[0:64, h2, :],
                    rcap[0:64, h2, :],
                )

            if qc >= 2:
                # DVE magic-seed + Newton reciprocal (0.26% max err), spread
                # over six j-offsets: ScalarE keeps only the exp stream in
                # the chunks where it is the binding engine.  t reuses ntd's
                # buffer and the result reuses u's (WAR-tracked).
                y0 = rc_pool.tile([128, 2, 512], FP32, tag="rc", name="y0")
                u = rc_pool.tile([128, 2, 512], FP32, tag="rc", name="u")
                deferred.extend([
                    (2, lambda: nc.vector.tensor_scalar(
                        y0[0:64, :, :].bitcast(I32), ntd[0:64, :, :].bitcast(I32),
                        RECIP_MAGIC, -1,
                        mybir.AluOpType.subtract, mybir.AluOpType.mult)),
                    (3, lambda: nc.vector.tensor_mul(
                        u[0:64, :, :], ntd[0:64, :, :], y0[0:64, :, :])),
                    (4, lambda: nc.vector.tensor_scalar(
                        ntd[0:64, :, :], u[0:64, :, :], -1.0, 2.0,
                        mybir.AluOpType.mult, mybir.AluOpType.add)),
                    (5, lambda: nc.vector.tensor_mul(
                        u[0:64, :, :], ntd[0:64, :, :], y0[0:64, :, :])),
                    (7, lambda: mul_unit(0, u)),
                    (8, lambda: mul_unit(1, u)),
                ])
            else:
                def norm(ntn=ntn, ntd=ntd):
                    rl = rc_pool.tile([128, 2, 512], FP32, tag="rc", name="rl")
                    rc = rc_pool.tile([128, 2, 512], FP32, tag="rc", name="rc")
                    nc.scalar.activation(rl[0:64, :, :], ntd[0:64, :, :],
                                         mybir.ActivationFunctionType.Ln)
                    nc.scalar.activation(rc[0:64, :, :], rl[0:64, :, :], EXP,
                                         scale=-1.0)
                    for h2 in range(2):
                        mul_unit(h2, rc)

                deferred.append((2, norm))

        # chunk schedule: per qc, refill the fill list and set the pacing rate
        chunks = [(qc, hp) for qc in range(QC) for hp in range(NHP)]
        for ci, (qc, hp) in enumerate(chunks):
            nxt = chunks[ci + 1] if ci + 1 < len(chunks) else None
            if hp == 0:
                # phase refill at each new query chunk
                new = []
                if qc == 0:
                    # kproj: tc0 of every head-pair first (needed from chunk
                    # (0, hp)); later tcs deferred to their query chunk
                    new += [
                        ((0, op, 0), lambda op=op: kproj_tc(op, 0))
                        for op in range(NHP)
                    ]
                    first = 4 if causal else TT
                    new += [
                        ((0, 0, tt), lambda tt=tt: vproj(tt)) for tt in range(first)
                    ]
                    if not causal:
                        new += [
                            ((0, op, 4 * tc), lambda op=op, tc=tc: kproj_tc(op, tc))
                            for tc in range(1, 4)
                            for op in range(NHP)
                        ]
                else:
                    if causal:
                        new += [
                            ((qc, op, 4 * qc), lambda op=op, qc=qc: kproj_tc(op, qc))
                            for op in range(NHP)
                        ]
                        new += [
                            ((qc, 0, tt), lambda tt=tt: vproj(tt))
                            for tt in range(4 * qc, 4 * qc + 4)
                        ]
                    # o-proj fills ride two query-chunks behind their data
                    # (a_sb[qc'] is complete once chunk (qc', 3) normalizes),
                    # loading the later, ACT-heavier chunks; the last four of
                    # oproj(qc2) are reserved to keep the PE warm through the
                    # final chunk's normalize + o-proj tail
                    if qc >= 2:
                        new += [
                            (None, lambda od=od, q=qc - 2: oproj_od(q, od))
                            for od in range(8)
                        ]
                    if qc == QC - 1:
                        tail_fills.extend(
                            (None, lambda od=od, q=qc - 1: oproj_od(q, od))
                            for od in range(4, 8)
                        )
                        new += [
                            (None, lambda od=od, q=qc - 1: oproj_od(q, od))
                            for od in range(4)
                        ]
                fills.extend(new)
                fills.sort(key=lambda f: (f[0] is None, f[0] or (0, 0, 0)))
                jmax = 4 * qc + 3 if causal else TT - 1
                ntiles = NHP * (jmax + 1)
                credit[0] = 0.0
                credit[1] = len(fills) / float(ntiles)
            attn(qc, hp, nxt)
        while deferred:
            deferred.pop(0)[1]()
        # reserved o-proj work rides the PE through the last chunk's
        # normalize so the array stays warm into the final o-proj
        while tail_fills:
            tail_fills.pop(0)[1]()
        while fills:
            fills.pop(0)[1]()

        # final chunk's o-proj: attention is over, so spread accumulators
        # across the free score banks too (6 chains in flight instead of 2)
        fin = [ps_s.tile([128, 2, 512], FP32, tag="ps_s", name=f"fin{i}") for i in range(2)]
        qc = QC - 1
        for od in range(8):
            if od < 4:
                oproj_od(qc, od, ps_ap=fin[od // 2][:, od % 2, :])
            else:
                oproj_od(qc, od)


_CACHE = {}


def _patched_act_tables(arch):
    """Make the combined Ln+Exp set the only one advertising Exp/Ln so the
    table-load pass picks it everywhere (one load, no set thrashing).  Set
    positions (= act_func_set_id) are preserved."""
    t = dict(_orig_act_tables(arch))
    name = "natural_log_exp_and_others"
    if name in t:
        exp_ln = {f for f in t[name] if f.name in ("Exp", "Ln")}
        t = {
            k: (v if k == name else (set(v) - exp_ln))
            for k, v in t.items()
        }
    return t


_orig_act_tables = bacc.get_activation_tables
bacc.get_activation_tables = _patched_act_tables


def _get_compiled(causal: bool):
    key = bool(causal)
    if key not in _CACHE:
        nc = bacc.Bacc("TRN2", target_bir_lowering=False, debug=False, num_devices=NCORES)
        _emit(nc, causal=key)
        nc.compile()
        _CACHE[key] = nc
    return _CACHE[key]


def make_in_maps(query, key, value, w_q, b_q, w_k, b_k, w_v, b_v, w_o, b_o):
    """Build the per-core input maps (host-side sharding + layout prep)."""
    in_maps = []
    # b_v folds into the output bias: softmax rows sum to 1, so
    # attn(V + b_v) = attn(V) + b_v, and (A + b_v) @ w_o.T = A @ w_o.T + w_o @ b_v.
    # b_k drops entirely: scores shift constant along k cancels in softmax.
    bo_eff = (b_o + w_o.astype(np.float64) @ b_v.astype(np.float64)).astype(np.float32)
    for c in range(NCORES):
        b, hg = divmod(c, 2)
        sl = slice(hg * DL, (hg + 1) * DL)
        bo_core = bo_eff if hg == 0 else np.zeros_like(bo_eff)
        in_maps.append(
            {
                "xq_t": np.ascontiguousarray(query[b].T).astype(BF16),
                "xk_t": np.ascontiguousarray(key[b].T).astype(BF16),
                "xv_t": np.ascontiguousarray(value[b].T).astype(BF16),
                "wq_p": np.ascontiguousarray(
                    w_q[sl, :].T.reshape(KT, 128, DL).transpose(1, 0, 2)).astype(BF16),
                "wk_p": np.ascontiguousarray(
                    w_k[sl, :].T.reshape(KT, 128, DL).transpose(1, 0, 2)).astype(BF16),
                "wv_p": np.ascontiguousarray(
                    w_v[sl, :].T.reshape(KT, 128, DL).transpose(1, 0, 2)).astype(BF16),
                "wo_p": np.ascontiguousarray(
                    w_o[:, sl].T.reshape(NHP, 128, D).transpose(1, 0, 2)).astype(BF16),
                "bq_t": np.ascontiguousarray(b_q[sl].reshape(4, 128).T).astype(np.float32),
                "bo_t": np.ascontiguousarray(bo_core.reshape(8, 128).T).astype(np.float32),
            }
        )
    return in_maps


def _mask_is_causal(mask):
    m = np.asarray(mask).reshape(S, S)
    return bool(np.array_equal(m, np.triu(np.ones((S, S), bool), k=1)))


def _mask_is_empty(mask):
    return not np.asarray(mask).any()


def kernel(query, key, value, mask, w_q, b_q, w_k, b_k, w_v, b_v, w_o, b_o, **_unused):
    query = np.asarray(query, np.float32)
    key = np.asarray(key, np.float32)
    value = np.asarray(value, np.float32)
    if _mask_is_causal(mask):
        causal = True
    elif _mask_is_empty(mask):
        causal = False
    else:
        raise NotImplementedError("only causal or empty masks are supported")

    nc = _get_compiled(causal)
    in_maps = make_in_maps(
        query, key, value,
        np.asarray(w_q, np.float32), np.asarray(b_q, np.float32),
        np.asarray(w_k, np.float32), np.asarray(b_k, np.float32),
        np.asarray(w_v, np.float32), np.asarray(b_v, np.float32),
        np.asarray(w_o, np.float32), np.asarray(b_o, np.float32),
    )
    res = bass_utils.run_bass_kernel_spmd(nc, in_maps, core_ids=list(range(NCORES)))
    out = np.empty((B, S, D), np.float32)
    for b in range(B):
        acc = res.results[2 * b] ["out_pt"] + res.results[2 * b + 1]["out_pt"]
        out[b] = acc.T
    return out
